# revision 26
# baseline (speedup 1.0000x reference)
"""NystromAttention Trainium2 Bass kernel (SPMD over 8 NeuronCores).

Sharding: (B,H)=96 slices flattened; core i takes slices [12i, 12i+12),
processed as 6 pairs stacked on the 128-partition dim.

v3 design (vs the 517us fp32 baseline, which was PE-bound at 93.5% with
fp32 4-cycle/row matmuls and 256B DMA descriptors):

- fp16 datapath for every BIG matmul (1 cycle/row on the PE instead of
  fp32's 4, plus fast-weight-load). fp32->fp16 cast happens inside the
  SWDGE ingest DMAs (gpsimd dma_start casts for free).
- The landmark->kernel_2->Newton-Schulz->W chain stays fp32: errors in
  the matrix being pseudo-inverted (and in the R/W product chain) are
  amplified by its conditioning; fp16 there costs 5e-2 rel error
  (measured in numpy emulation), fp32 chain + fp16 big path = 1.3e-4.
  These are all tiny 64x64 matmuls, so the fp32 4-cycle cost is small.
- Host-side DRAM staging: Q/K stored pair-interleaved [48, S, 128]
  (= [Q_a[s] | Q_b[s]] per row) and V stored [96, S, 65] with the mask
  appended as column 64. Ingest DMA runs become 2KB contiguous on the
  DRAM side (375 GB/s class vs 213 GB/s at 256B runs), and the
  [a|b]-fused transpose chunks / [V|mask] G-matmul lhsT become single
  contiguous windows (walrus wants 1-free-dim matmul operands).
- Quad-interleaved s-permutation: within each 512-row block, SBUF
  column 128*t + p holds DRAM row 4*p + t. Carried through all
  intermediate tensors and undone in the output store (1KB store runs).
- Landmark segment sums fused into the transpose matmuls:
  rhs = [I128 | ACOL8] (N=136), partials split off to an fp32 strip
  during the PSUM->SBUF copy and summed on DVE. Kills the separate
  per-chunk landmark matmul + its duplicate weight load.
- r3 (kernel_3 row sums) fused into the G matmuls via the 65-column
  [V | mask] lhsT. Kills the per-chunk mask-row matmuls.

All softmaxes skip max-subtraction (logits are ~N(0, 0.125)). Scales
are folded into the ACT exp. Landmarks are kept as segment SUMS (the
/64 is folded into the exp scale).

Newton-Schulz pseudo-inverse is reformulated on N = (1/c) Km^T Km,
which is symmetric, so the whole iteration needs no transposes:
  N_{k+1} = 0.25 N_k Qp(N_k),  Qp(X) = 13I - 15X + 7X^2 - X^3
  R = prod_k 0.25 Qp(N_k)  =>  Vi6 = (1/c) R Km^T
  W = Vi6 @ (diag(1/r3) G) = (1/c) R @ (Km^T G~)
The reference's init scale c = max over ALL (b,h) of colsums of
kernel_2 couples the shards; we compute c exactly on the host (cheap
numpy reduction producing one scalar) and pass 1/c as a tiny input.
"""

import numpy as np

B, H, S, D, L = 8, 12, 4096, 64, 64
NCORES = 8
PER_CORE = (B * H) // NCORES      # 12 slices
NPAIRS = PER_CORE // 2            # 6
NBLK = S // 512                   # 8 blocks of 512 rows
NCHUNK = S // 128                 # 32 chunks (bb, t)
SCALE2 = 0.125                    # (d^-1/4)^2
EXP_SCALE_SL = SCALE2 / 64.0      # for S1, S3 logits (one landmark-sum side)
EXP_SCALE_S2 = SCALE2 / 4096.0    # for S2 logits (two landmark-sum sides)

# fp16 consts column layout
C_I128 = 0        # [128,128] identity (I|ACOL must be adjacent)
C_ACOL = 128      # [128,8] landmark indicator cols (16-row bands)
C_ONES = 136      # [128,1] ones column (r3 reduction lhsT)
C_NCOLS = 137
# fp32 consts
C32_I13 = 0       # [128,64] 13*[I64;I64]
C32_P15 = 64      # [128,128] 15*I
C32_M7 = 192      # [128,128] -7*I
C32_I65 = 320     # [128,65] I65 in rows 0:65
C32_NCOLS = 385

_PROG_CACHE = {}


def _make_consts():
    C = np.zeros((128, C_NCOLS), np.float16)
    I128 = np.eye(128, dtype=np.float16)
    C[:, C_I128:C_I128 + 128] = I128
    for j in range(8):
        C[16 * j:16 * j + 16, C_ACOL + j] = 1.0
    C[:, C_ONES] = 1.0
    C32 = np.zeros((128, C32_NCOLS), np.float32)
    I64 = np.eye(64, dtype=np.float32)
    C32[0:64, C32_I13:C32_I13 + 64] = 13.0 * I64
    C32[64:128, C32_I13:C32_I13 + 64] = 13.0 * I64
    I128f = np.eye(128, dtype=np.float32)
    C32[:, C32_P15:C32_P15 + 128] = 15.0 * I128f
    C32[:, C32_M7:C32_M7 + 128] = -7.0 * I128f
    C32[0:65, C32_I65:C32_I65 + 65] = np.eye(65, dtype=np.float32)
    return C, C32


def _host_global_c(Q, K, mask):
    """Exact global max of kernel_2 column-sums (one fp32 scalar)."""
    scale = np.float32(1.0 / np.sqrt(np.sqrt(D)))
    if mask.min() >= 1.0 and mask.max() <= 1.0:
        Qs = Q
        Ks = K
    else:
        m = mask[:, None, :, None].astype(np.float32)
        Qs = Q * m
        Ks = K * m
    seg = S // L
    Q_l = Qs.reshape(B, H, L, seg, D).mean(axis=-2, dtype=np.float32) * scale
    K_l = Ks.reshape(B, H, L, seg, D).mean(axis=-2, dtype=np.float32) * scale
    s2 = np.einsum('bhld,bhmd->bhlm', Q_l, K_l).astype(np.float32)
    s2 -= s2.max(axis=-1, keepdims=True)
    e = np.exp(s2, dtype=np.float32)
    k2 = e / e.sum(axis=-1, keepdims=True, dtype=np.float32)
    return np.float32(k2.sum(axis=-2, dtype=np.float32).max())


def _build_program(npairs=NPAIRS, debug=False, ones_mask=True):
    import concourse.bacc as bacc
    import concourse.mybir as mybir
    import concourse.tile as tile
    from concourse.bass import ds

    f32 = mybir.dt.float32
    f16 = mybir.dt.float16
    AF = mybir.ActivationFunctionType
    AX = mybir.AxisListType

    per_core = npairs * 2
    nc = bacc.Bacc("TRN2", target_bir_lowering=False, debug=False)
    qd = nc.dram_tensor("q", [npairs, S, 128], f32, kind="ExternalInput").ap()
    kd = nc.dram_tensor("k", [npairs, S, 128], f32, kind="ExternalInput").ap()
    vd = nc.dram_tensor("v", [per_core, S, 68], f32, kind="ExternalInput").ap()
    rcd = nc.dram_tensor("rc", [128, 1], f32, kind="ExternalInput").ap()
    cd = nc.dram_tensor("c", [128, C_NCOLS], f16, kind="ExternalInput").ap()
    cd32 = nc.dram_tensor("c32", [128, C32_NCOLS], f32,
                          kind="ExternalInput").ap()
    xd = nc.dram_tensor("x", [per_core, S, D], f32, kind="ExternalOutput").ap()
    if debug:
        dbg = {
            "dbg_qts": nc.dram_tensor("dbg_qts", [128, 4096], f16,
                                      kind="ExternalOutput").ap(),
            "dbg_pq": nc.dram_tensor("dbg_pq", [128, 256], f32,
                                     kind="ExternalOutput").ap(),
            "dbg_lmq": nc.dram_tensor("dbg_lmq", [128, 64], f32,
                                      kind="ExternalOutput").ap(),
            "dbg_lmk": nc.dram_tensor("dbg_lmk", [128, 64], f32,
                                      kind="ExternalOutput").ap(),
            "dbg_km": nc.dram_tensor("dbg_km", [128, 64], f32,
                                     kind="ExternalOutput").ap(),
            "dbg_gts": nc.dram_tensor("dbg_gts", [128, 128], f32,
                                      kind="ExternalOutput").ap(),
            "dbg_gt": nc.dram_tensor("dbg_gt", [128, 64], f32,
                                     kind="ExternalOutput").ap(),
            "dbg_wbd": nc.dram_tensor("dbg_wbd", [128, 130], f16,
                                      kind="ExternalOutput").ap(),
            "dbg_e1t": nc.dram_tensor("dbg_e1t", [128, 512], f16,
                                      kind="ExternalOutput").ap(),
            "dbg_e3t": nc.dram_tensor("dbg_e3t", [128, 512], f16,
                                      kind="ExternalOutput").ap(),
            "dbg_rst": nc.dram_tensor("dbg_rst", [128, 64], f32,
                                      kind="ExternalOutput").ap(),
        }

    with tile.TileContext(nc) as tc:
        with (
            tc.tile_pool(name="cst", bufs=1) as cpool,
            tc.tile_pool(name="bigT", bufs=2) as bigT,
            tc.tile_pool(name="med", bufs=3) as med,
            tc.tile_pool(name="sml", bufs=2) as sml,
            tc.tile_pool(name="psA", bufs=3, space="PSUM") as psA,
            tc.tile_pool(name="psB", bufs=2, space="PSUM") as psB,
            tc.tile_pool(name="psC", bufs=3, space="PSUM") as psC,
        ):
            cst = cpool.tile([128, C_NCOLS], f16)
            nc.sync.dma_start(out=cst, in_=cd)
            cst32 = cpool.tile([128, C32_NCOLS], f32)
            nc.sync.dma_start(out=cst32, in_=cd32)
            rcb = cpool.tile([128, 1], f32)
            nc.sync.dma_start(out=rcb, in_=rcd)
            IA = cst[:, C_I128:C_I128 + 136]     # [I128 | ACOL8] fp16
            ONES1 = cst[:, C_ONES:C_ONES + 1]    # [128,1] ones fp16
            I13 = cst32[:, C32_I13:C32_I13 + 64]
            P15 = cst32[:, C32_P15:C32_P15 + 128]
            M7 = cst32[:, C32_M7:C32_M7 + 128]
            I65 = cst32[0:65, C32_I65:C32_I65 + 65]

            pending_x = []

            def emit_x(p, a, b, e1t, wbd):
                # xo cols = (h 2, bb 2, t 4, d 64); store runs 1KB both sides
                for u in range(4):  # 1024-row store units (2 blocks each)
                    xo = med.tile([128, 1024], f32, tag="xo",
                                  name=f"xo{p}_{u}")
                    xov = xo.rearrange("p (h bb t d) -> p h bb t d",
                                       h=2, bb=2, t=4)
                    for k in range(4):  # 2 chunks per psum bank
                        ps_x = psC.tile([128, 512], f32, tag="xinv",
                                        name=f"psx{p}_{u}_{k}")
                        for r in range(2):
                            c = 8 * u + 2 * k + r
                            nc.tensor.matmul(
                                ps_x[:, ds(130 * r, 130)],
                                e1t[:, ds(128 * c, 128)], wbd,
                                start=True, stop=True,
                                skip_group_check=True)
                        psxv = ps_x[:, 0:260].rearrange(
                            "p (r h w) -> p r h w", r=2, h=2)
                        rr = sml.tile([128, 4], f32, tag="rr",
                                      name=f"rr{p}_{u}_{k}")
                        rrv = rr.rearrange("p (r h) -> p r h", r=2)
                        nc.vector.reciprocal(
                            rrv, psxv[:, :, :, 64:65]
                            .rearrange("p r h one -> p r (h one)"))
                        bb, t0 = (2 * k) // 4, (2 * k) % 4
                        nc.vector.tensor_mul(
                            xov[:, :, bb, t0:t0 + 2, :],
                            psxv[:, :, :, 0:64]
                            .rearrange("p r h d -> p h r d"),
                            rrv.rearrange("p r h -> p h r")[:, :, :, None]
                            .broadcast_to([128, 2, 2, 64]))
                    for h, sl in ((0, a), (1, b)):
                        nc.sync.dma_start(
                            out=xd[sl, ds(1024 * u, 1024), :]
                            .rearrange("(bb p t) d -> p bb (t d)",
                                       bb=2, p=128),
                            in_=xo.rearrange("p (h c) -> p h c", h=2)[:, h]
                            .rearrange("p (bb c) -> p bb c", bb=2))

            for p in range(npairs):
                a, b = 2 * p, 2 * p + 1

                # ---------- ingest: SWDGE cast fp32 -> fp16 ----------
                # ntq/ntk cols = blk(8) x t(4) x (h d)(128); DRAM runs 2KB.
                # Within block bb, SBUF chunk col 128*t + p <-> row 4*p + t.
                ntq = bigT.tile([128, 4096], f16, tag="ntq", name=f"ntq{p}")
                ntk = bigT.tile([128, 4096], f16, tag="ntk", name=f"ntk{p}")
                for srcd, nt in ((qd, ntq), (kd, ntk)):
                    nc.gpsimd.dma_start(
                        out=nt.rearrange("p (bb c) -> p bb c", bb=NBLK),
                        in_=srcd[p].rearrange("(bb p t) c -> p bb (t c)",
                                              bb=NBLK, p=128))
                # vva/vvb cols = blk(8) x t(4) x (d|mask)(65); DRAM runs ~1KB.
                vva = bigT.tile([128, 2176], f16, tag="vva", name=f"vva{p}")
                vvb = bigT.tile([128, 2176], f16, tag="vvb", name=f"vvb{p}")
                for sl, vv in ((a, vva), (b, vvb)):
                    nc.gpsimd.dma_start(
                        out=vv.rearrange("p (bb c) -> p bb c", bb=NBLK),
                        in_=vd[sl].rearrange("(bb p t) c -> p bb (t c)",
                                             bb=NBLK, p=128))

                # ---------- T phase: fused transpose + landmark sums ----
                # chunk c = 4*bb + t: lhsT = nt[:, 128c:+128] ([s, (h d)]),
                # rhs = [I128 | ACOL8] -> psum [128, 136]: cols 0:128 =
                # chunk.T (qt piece), cols 128:136 = 16-row-band sums
                # (landmark partials for (bb, t, j)). The copy back splits
                # the transpose part (fp16, to qts/kts) from the partials
                # (fp32 strip pq/pk).
                qts = bigT.tile([128, 4096], f16, tag="qts", name=f"qts{p}")
                kts = bigT.tile([128, 4096], f16, tag="kts", name=f"kts{p}")
                pq = sml.tile([128, 256], f32, tag="pq", name=f"pq{p}")
                pk = sml.tile([128, 256], f32, tag="pk", name=f"pk{p}")
                lmq = sml.tile([128, 64], f32, tag="lmq", name=f"lmq{p}")
                lmk = sml.tile([128, 64], f32, tag="lmk", name=f"lmk{p}")
                for ti, (nt, dst, pstrip, lm) in enumerate(
                        ((ntq, qts, pq, lmq), (ntk, kts, pk, lmk))):
                    for g in range(11):  # 3 chunks per psum bank (last: 2)
                        n_in_g = 3 if g < 10 else 2
                        pst = psA.tile([128, 512], f32, tag="bigps",
                                       name=f"pst{p}_{ti}_{g}")
                        for k in range(n_in_g):
                            c = 3 * g + k
                            nc.tensor.matmul(
                                pst[:, ds(136 * k, 136)],
                                nt[:, ds(128 * c, 128)], IA,
                                start=True, stop=True,
                                skip_group_check=True)
                        pstv = pst[:, 0:136 * n_in_g] \
                            .rearrange("p (k w) -> p k w", w=136)
                        tcp = dst[:, ds(384 * g, 128 * n_in_g)] \
                            .rearrange("p (k w) -> p k w", w=128)
                        pcp = pstrip[:, ds(24 * g, 8 * n_in_g)] \
                            .rearrange("p (k w) -> p k w", w=8)
                        if (ti + g) % 2 == 0:
                            nc.vector.tensor_copy(tcp,
                                                  pstv[:, 0:n_in_g, 0:128])
                            nc.scalar.copy(out=pcp,
                                           in_=pstv[:, 0:n_in_g, 128:136])
                        else:
                            nc.scalar.copy(out=tcp,
                                           in_=pstv[:, 0:n_in_g, 0:128])
                            nc.vector.tensor_copy(pcp,
                                                  pstv[:, 0:n_in_g, 128:136])
                    # landmark partials: pstrip[:, 8c : 8c+8] for c =
                    # (bb, t); sum over t on DVE (3 adds). l = 8*bb + j.
                    lv = pstrip.rearrange("p (bb t j) -> p bb t j",
                                          bb=NBLK, t=4)
                    t01 = sml.tile([128, 64], f32, tag="t01",
                                   name=f"t01{p}_{ti}")
                    t01v = t01.rearrange("p (bb j) -> p bb j", bb=NBLK)
                    nc.vector.tensor_add(t01v, lv[:, :, 0, :], lv[:, :, 1, :])
                    t23 = sml.tile([128, 64], f32, tag="t23",
                                   name=f"t23{p}_{ti}")
                    t23v = t23.rearrange("p (bb j) -> p bb j", bb=NBLK)
                    nc.vector.tensor_add(t23v, lv[:, :, 2, :], lv[:, :, 3, :])
                    nc.vector.tensor_add(lm, t01, t23)

                # blockdiag landmark tiles, fp32 (S2/NS path) + fp16 casts
                # (E1/E3 logits path)
                bdq = sml.tile([128, 128], f32, tag="bdq", name=f"bdq{p}")
                bdk = sml.tile([128, 128], f32, tag="bdk", name=f"bdk{p}")
                for bd, lm in ((bdq, lmq), (bdk, lmk)):
                    nc.gpsimd.memset(bd[0:64, 64:128], 0.0)
                    nc.gpsimd.memset(bd[64:128, 0:64], 0.0)
                    nc.vector.tensor_copy(bd[0:64, 0:64], lm[0:64, :])
                    nc.vector.tensor_copy(bd[64:128, 64:128], lm[64:128, :])
                bdq16 = sml.tile([128, 128], f16, tag="bdq16",
                                 name=f"bdq16{p}")
                bdk16 = sml.tile([128, 128], f16, tag="bdk16",
                                 name=f"bdk16{p}")
                nc.vector.tensor_copy(bdq16, bdq)
                nc.scalar.copy(out=bdk16, in_=bdk)

                # ---------- S2 / kernel_2 (fp32) ----------
                ps_s2 = psC.tile([128, 512], f32, tag="xinv", name=f"pss2{p}")
                nc.tensor.matmul(ps_s2[0:64, 0:64], bdq[0:64, 0:64],
                                 bdk[0:64, 0:64], start=True, stop=True,
                                 tile_position=(0, 0))
                nc.tensor.matmul(ps_s2[64:128, 0:64], bdq[64:128, 64:128],
                                 bdk[64:128, 64:128], start=True, stop=True,
                                 tile_position=(64, 64))
                e2 = sml.tile([128, 64], f32, tag="e2", name=f"e2{p}")
                nc.scalar.activation(e2, ps_s2[:, 0:64], AF.Exp,
                                     scale=EXP_SCALE_S2)
                r2 = sml.tile([128, 1], f32, tag="r2", name=f"r2{p}")
                nc.vector.reduce_sum(r2, e2, axis=AX.X)
                nc.vector.reciprocal(r2, r2)
                km = sml.tile([128, 64], f32, tag="km", name=f"km{p}")
                nc.vector.tensor_mul(km, e2, r2.broadcast_to([128, 64]))

                # ---------- N0 = (1/c) Km^T Km (fp32) ----------
                ps_n0 = psC.tile([128, 512], f32, tag="xinv", name=f"psn0{p}")
                nc.tensor.matmul(ps_n0[0:64, 0:64], km[0:64, :], km[0:64, :],
                                 start=True, stop=True, tile_position=(0, 0))
                nc.tensor.matmul(ps_n0[64:128, 0:64], km[64:128, :],
                                 km[64:128, :], start=True, stop=True,
                                 tile_position=(64, 64))
                n_st = sml.tile([128, 64], f32, tag="nst", name=f"n0{p}")
                nc.vector.tensor_mul(n_st, ps_n0[:, 0:64],
                                     rcb.broadcast_to([128, 64]))

                # ---------- Newton-Schulz iteration body (fp32) ----------
                # Emitted INTERLEAVED with the E3/G and E1 groups below so
                # the PE always has independent fp16 work queued between the
                # serially-dependent NS matmul->DVE->matmul round trips
                # (otherwise the PE idles and HAM re-throttles it to 1.2GHz).
                ns = {"n": n_st, "r": None}

                def ns_iter(it):
                    n_st, r_st = ns["n"], ns["r"]
                    ps_sq = psC.tile([128, 512], f32, tag="xinv",
                                     name=f"psq{p}_{it}")
                    nc.tensor.matmul(ps_sq[0:64, 0:64], n_st[0:64, :],
                                     n_st[0:64, :], start=True, stop=True,
                                     tile_position=(0, 0))
                    nc.tensor.matmul(ps_sq[64:128, 0:64], n_st[64:128, :],
                                     n_st[64:128, :], start=True, stop=True,
                                     tile_position=(64, 64))
                    n2 = sml.tile([128, 64], f32, tag="n2", name=f"n2{p}_{it}")
                    nc.vector.tensor_copy(n2, ps_sq[:, 0:64])
                    # Qp' = 15N - 7N^2 + N^3; only N^3 on the PE (quads),
                    # the diagonal-matrix terms are cheaper as fused DVE ops
                    # than as P15/M7 fp32 matmuls (each of those cost a
                    # double LDWEIGHTS + 4-cycle/row matmul).
                    ps_qp = psC.tile([128, 512], f32, tag="xinv",
                                     name=f"psqp{p}_{it}")
                    nc.tensor.matmul(ps_qp[0:64, 0:64], n_st[0:64, :],
                                     n2[0:64, :], start=True, stop=True,
                                     tile_position=(0, 0),
                                     skip_group_check=True)
                    nc.tensor.matmul(ps_qp[64:128, 0:64], n_st[64:128, :],
                                     n2[64:128, :], start=True, stop=True,
                                     tile_position=(64, 64),
                                     skip_group_check=True)
                    # u = 15*N + N^3
                    u_t = sml.tile([128, 64], f32, tag="ut",
                                   name=f"u{p}_{it}")
                    nc.vector.scalar_tensor_tensor(
                        u_t, n_st, 15.0, ps_qp[:, 0:64],
                        op0=mybir.AluOpType.mult, op1=mybir.AluOpType.add)
                    # v = -7*N^2 + u
                    v_t = sml.tile([128, 64], f32, tag="vt",
                                   name=f"v{p}_{it}")
                    nc.vector.scalar_tensor_tensor(
                        v_t, n2, -7.0, u_t,
                        op0=mybir.AluOpType.mult, op1=mybir.AluOpType.add)
                    qp = sml.tile([128, 64], f32, tag="qp", name=f"qp{p}_{it}")
                    nc.vector.tensor_sub(qp, v_t, I13)
                    if it == 0:
                        r_new = sml.tile([128, 64], f32, tag="rst",
                                         name=f"r{p}_{it}")
                        nc.vector.tensor_scalar_mul(r_new, qp, -0.25)
                    else:
                        ps_r = psC.tile([128, 512], f32, tag="xinv",
                                        name=f"psr{p}_{it}")
                        nc.tensor.matmul(ps_r[0:64, 0:64], r_st[0:64, :],
                                         qp[0:64, :], start=True, stop=True,
                                         tile_position=(0, 0))
                        nc.tensor.matmul(ps_r[64:128, 0:64], r_st[64:128, :],
                                         qp[64:128, :], start=True, stop=True,
                                         tile_position=(64, 64))
                        r_new = sml.tile([128, 64], f32, tag="rst",
                                         name=f"r{p}_{it}")
                        nc.vector.tensor_scalar_mul(r_new, ps_r[:, 0:64],
                                                    -0.25)
                    ns["r"] = r_new
                    if it < 5:
                        ps_nn = psC.tile([128, 512], f32, tag="xinv",
                                         name=f"psnn{p}_{it}")
                        nc.tensor.matmul(ps_nn[0:64, 0:64], n_st[0:64, :],
                                         qp[0:64, :], start=True, stop=True,
                                         tile_position=(0, 0))
                        nc.tensor.matmul(ps_nn[64:128, 0:64], n_st[64:128, :],
                                         qp[64:128, :], start=True, stop=True,
                                         tile_position=(64, 64))
                        n_new = sml.tile([128, 64], f32, tag="nst",
                                         name=f"n{p}_{it}")
                        nc.vector.tensor_scalar_mul(n_new, ps_nn[:, 0:64],
                                                    -0.25)
                        ns["n"] = n_new

                # ---------- E3^T and G^T = [V|m]^T E3~^T (fp16 mms) -----
                # One accumulator bank PER SLICE: start=True clears the
                # has_written bits of the whole bank on the written
                # partitions, so two interleaved accumulation streams on the
                # same partitions of one bank lose the first stream's c=0
                # contribution (measured: exactly-missing-chunk-0).
                # Rows 0:64 = G^T, row 64 = r3.
                # previous pair's X phase drops in here: its DVE-heavy
                # normalize/store overlaps this pair's PE-heavy E3/E1 stream
                if pending_x:
                    emit_x(*pending_x.pop(0))

                ps_ga = psB.tile([128, 512], f32, tag="gacc", name=f"psga{p}")
                ps_gb = psB.tile([128, 512], f32, tag="gacc", name=f"psgb{p}")
                e1t = bigT.tile([128, 4096], f16, tag="e1t", name=f"e1t{p}")
                for g in range(8):
                    ps_e3 = psA.tile([128, 512], f32, tag="bigps",
                                     name=f"pse3{p}_{g}")
                    for ci in range(4):
                        c = 4 * g + ci
                        nc.tensor.matmul(ps_e3[:, ds(128 * ci, 128)],
                                         kts[:, ds(128 * c, 128)], bdq16,
                                         start=True, stop=True,
                                         skip_group_check=True)
                    e3t = med.tile([128, 512], f16, tag="e3t",
                                   name=f"e3t{p}_{g}")
                    nc.scalar.activation(e3t, ps_e3, AF.Exp,
                                         scale=EXP_SCALE_SL)
                    if debug and p == 0 and g == 0:
                        nc.sync.dma_start(out=dbg["dbg_e3t"], in_=e3t)
                    gw = 64 if ones_mask else 65
                    for ci in range(4):
                        c = 4 * g + ci
                        first, last = (c == 0), (c == NCHUNK - 1)
                        vcol = 272 * g + 68 * ci  # (bb=g, t=ci) chunk
                        nc.tensor.matmul(
                            ps_ga[0:gw, 0:64],
                            vva[:, ds(vcol, gw)],
                            e3t[:, ds(128 * ci, 64)],
                            start=first, stop=last,
                            tile_position=(0, 0), skip_group_check=True)
                        nc.tensor.matmul(
                            ps_gb[0:gw, 0:64],
                            vvb[:, ds(vcol, gw)],
                            e3t[:, ds(128 * ci + 64, 64)],
                            start=first, stop=last,
                            tile_position=(0, 0), skip_group_check=True)
                    if ones_mask:
                        # r3 partials for BOTH slices in one N=512 matmul:
                        # out[0, (ci, h, l)] += column sums of e3t; lives on
                        # partition 64 of ps_ga (disjoint from G's 0:64).
                        nc.tensor.matmul(
                            ps_ga[64:65, 0:512], ONES1, e3t,
                            start=(g == 0), stop=(g == 7),
                            tile_position=(0, 64), skip_group_check=True)
                    # ---- E1 group g (independent fp16 work) ----
                    ps_s1 = psA.tile([128, 512], f32, tag="bigps",
                                     name=f"pss1{p}_{g}")
                    nc.tensor.matmul(ps_s1, bdk16, qts[:, ds(512 * g, 512)],
                                     start=True, stop=True)
                    nc.scalar.activation(e1t[:, ds(512 * g, 512)], ps_s1,
                                         AF.Exp, scale=EXP_SCALE_SL)
                    # ---- NS iteration g, hidden behind the fp16 stream ----
                    if g < 6:
                        ns_iter(g)
                r_st = ns["r"]
                # transpose G^T (+r3 row) back to [l, (d|r3)], fp32
                gts = sml.tile([128, 128], f32, tag="gts", name=f"gts{p}")
                if ones_mask:
                    nc.vector.tensor_copy(gts[0:64, 0:64], ps_ga[0:64, 0:64])
                    nc.vector.tensor_copy(gts[0:64, 64:128],
                                          ps_gb[0:64, 0:64])
                    # r3 row: copy the partial row out of PSUM first (the
                    # verifier rejects tensor_tensor with both operands in
                    # one PSUM bank), then 3 SBUF adds over ci and a
                    # 1-partition ACT copy into gts row 64.
                    r3tmp = sml.tile([128, 512 + 128], f32, tag="r3t",
                                     name=f"r3t{p}")
                    nc.scalar.copy(out=r3tmp[64:65, 0:512],
                                   in_=ps_ga[64:65, 0:512])
                    r3v = r3tmp[:, 0:512].rearrange(
                        "p (ci c) -> p ci c", ci=4)
                    nc.vector.tensor_add(r3tmp[:, 512:640],
                                         r3v[:, 0, :], r3v[:, 1, :])
                    nc.vector.tensor_add(r3tmp[:, 512:640],
                                         r3tmp[:, 512:640], r3v[:, 2, :])
                    nc.vector.tensor_add(gts[64:65, 0:128],
                                         r3tmp[64:65, 512:640],
                                         r3v[64:65, 3, :])
                else:
                    nc.vector.tensor_copy(gts[0:65, 0:64], ps_ga[0:65, 0:64])
                    nc.vector.tensor_copy(gts[0:65, 64:128],
                                          ps_gb[0:65, 0:64])
                ps_g2 = psC.tile([128, 512], f32, tag="xinv", name=f"psg2{p}")
                nc.tensor.matmul(ps_g2[:, 0:65], gts[0:65, 0:128], I65,
                                 start=True, stop=True)
                r3r = sml.tile([128, 1], f32, tag="r3", name=f"r3{p}")
                nc.vector.reciprocal(r3r, ps_g2[:, 64:65])
                gt = sml.tile([128, 64], f32, tag="gt", name=f"gt{p}")
                nc.vector.tensor_mul(gt, ps_g2[:, 0:64],
                                     r3r.broadcast_to([128, 64]))

                # ---------- W = (1/c) R @ (Km^T G~) (fp32) ----------
                ps_kg = psC.tile([128, 512], f32, tag="xinv", name=f"pskg{p}")
                nc.tensor.matmul(ps_kg[0:64, 0:64], km[0:64, :], gt[0:64, :],
                                 start=True, stop=True, tile_position=(0, 0))
                nc.tensor.matmul(ps_kg[64:128, 0:64], km[64:128, :],
                                 gt[64:128, :], start=True, stop=True,
                                 tile_position=(64, 64))
                kg = sml.tile([128, 64], f32, tag="kg", name=f"kg{p}")
                nc.vector.tensor_copy(kg, ps_kg[:, 0:64])
                ps_w = psC.tile([128, 512], f32, tag="xinv", name=f"psw{p}")
                nc.tensor.matmul(ps_w[0:64, 0:64], r_st[0:64, :], kg[0:64, :],
                                 start=True, stop=True, tile_position=(0, 0))
                nc.tensor.matmul(ps_w[64:128, 0:64], r_st[64:128, :],
                                 kg[64:128, :], start=True, stop=True,
                                 tile_position=(64, 64))
                wbd = sml.tile([128, 130], f16, tag="wbd", name=f"wbd{p}")
                nc.gpsimd.memset(wbd[0:64, 65:130], 0.0)
                nc.gpsimd.memset(wbd[64:128, 0:65], 0.0)
                nc.gpsimd.memset(wbd[0:64, 64:65], 1.0)
                nc.gpsimd.memset(wbd[64:128, 129:130], 1.0)
                nc.vector.tensor_mul(wbd[0:64, 0:64], ps_w[0:64, 0:64],
                                     rcb[0:64, :].broadcast_to([64, 64]))
                nc.vector.tensor_mul(wbd[64:128, 65:129], ps_w[64:128, 0:64],
                                     rcb[64:128, :].broadcast_to([64, 64]))

                if debug and p == 0:
                    nc.sync.dma_start(out=dbg["dbg_qts"], in_=qts)
                    nc.sync.dma_start(out=dbg["dbg_pq"], in_=pq)
                    nc.sync.dma_start(out=dbg["dbg_lmq"], in_=lmq)
                    nc.sync.dma_start(out=dbg["dbg_lmk"], in_=lmk)
                    nc.sync.dma_start(out=dbg["dbg_km"], in_=km)
                    nc.sync.dma_start(out=dbg["dbg_gts"], in_=gts)
                    nc.sync.dma_start(out=dbg["dbg_gt"], in_=gt)
                    nc.sync.dma_start(out=dbg["dbg_wbd"], in_=wbd)
                    nc.sync.dma_start(out=dbg["dbg_e1t"],
                                      in_=e1t[:, 0:512])
                    nc.sync.dma_start(out=dbg["dbg_rst"], in_=r_st)

                # ---------- X = diag(1/r1) E1 W (fp16 mms) ----------
                # Deferred one pair: pair p's X is emitted during pair p+1's
                # T/E3 stream so its DVE-heavy normalize+store overlaps the
                # next pair's PE-heavy phases (and vice versa).
                pending_x.append((p, a, b, e1t, wbd))
            while pending_x:
                emit_x(*pending_x.pop(0))
    return nc


def _get_program(npairs=NPAIRS, debug=False, ones_mask=True):
    key = (npairs, debug, ones_mask)
    if key not in _PROG_CACHE:
        nc = _build_program(npairs, debug, ones_mask)
        if not nc.is_finalized():
            nc.finalize()  # Bacc defers register allocation until finalize
        _PROG_CACHE[key] = nc
    return _PROG_CACHE[key]


def run(inputs, trace=False, trace_kwargs=None, debug=False):
    from concourse import bass_utils
    Q, K, V, mask = (np.asarray(inputs["Q"], np.float32),
                     np.asarray(inputs["K"], np.float32),
                     np.asarray(inputs["V"], np.float32),
                     np.asarray(inputs["mask"], np.float32))
    ones_mask = bool(mask.min() >= 1.0 and mask.max() <= 1.0)
    rc = np.full((128, 1), 1.0 / _host_global_c(Q, K, mask), np.float32)
    consts16, consts32 = _make_consts()

    if ones_mask:
        Qm, Km, Vm = Q, K, V
    else:
        m = mask[:, None, :, None].astype(np.float32)
        Qm, Km, Vm = Q * m, K * m, V * m

    npair_tot = (B * H) // 2
    # pair-interleaved [48, S, 128]: row s = [T_a[s] | T_b[s]]
    Qp = np.ascontiguousarray(
        Qm.reshape(npair_tot, 2, S, D).transpose(0, 2, 1, 3)
        .reshape(npair_tot, S, 128))
    Kp = np.ascontiguousarray(
        Km.reshape(npair_tot, 2, S, D).transpose(0, 2, 1, 3)
        .reshape(npair_tot, S, 128))
    # V with mask appended as column 64, padded to 68 so each DRAM row
    # is 272B (16B-aligned descriptor starts): [96, S, 68]
    Vx = np.zeros((B * H, S, 68), np.float32)
    Vx[:, :, :64] = Vm.reshape(B * H, S, D)
    Vx[:, :, 64] = np.broadcast_to(mask[:, None, :], (B, H, S)) \
        .reshape(B * H, S)

    nc = _get_program(debug=debug, ones_mask=ones_mask)
    in_maps = []
    for c in range(NCORES):
        in_maps.append({
            "q": Qp[c * NPAIRS:(c + 1) * NPAIRS],
            "k": Kp[c * NPAIRS:(c + 1) * NPAIRS],
            "v": Vx[c * PER_CORE:(c + 1) * PER_CORE],
            "rc": rc,
            "c": consts16,
            "c32": consts32,
        })
    res = bass_utils.run_bass_kernel_spmd(
        nc, in_maps, core_ids=list(range(NCORES)), trace=trace,
        **(trace_kwargs or {}))
    X = np.concatenate([r["x"] for r in res.results], axis=0)
    return X.reshape(B, H, S, D), res


def kernel(**inputs):
    X, _ = run(inputs, trace=False)
    return X


if __name__ == "__main__":
    # quick build check
    prog = _get_program()
    print("built ok")


# revision 27
# speedup vs baseline: 1.0167x; 1.0167x over previous
"""NystromAttention Trainium2 Bass kernel (SPMD over 8 NeuronCores).

Sharding: (B,H)=96 slices flattened; core i takes slices [12i, 12i+12),
processed as 6 pairs stacked on the 128-partition dim.

v3 design (vs the 517us fp32 baseline, which was PE-bound at 93.5% with
fp32 4-cycle/row matmuls and 256B DMA descriptors):

- fp16 datapath for every BIG matmul (1 cycle/row on the PE instead of
  fp32's 4, plus fast-weight-load). fp32->fp16 cast happens inside the
  SWDGE ingest DMAs (gpsimd dma_start casts for free).
- The landmark->kernel_2->Newton-Schulz->W chain stays fp32: errors in
  the matrix being pseudo-inverted (and in the R/W product chain) are
  amplified by its conditioning; fp16 there costs 5e-2 rel error
  (measured in numpy emulation), fp32 chain + fp16 big path = 1.3e-4.
  These are all tiny 64x64 matmuls, so the fp32 4-cycle cost is small.
- Host-side DRAM staging: Q/K stored pair-interleaved [48, S, 128]
  (= [Q_a[s] | Q_b[s]] per row) and V stored [96, S, 65] with the mask
  appended as column 64. Ingest DMA runs become 2KB contiguous on the
  DRAM side (375 GB/s class vs 213 GB/s at 256B runs), and the
  [a|b]-fused transpose chunks / [V|mask] G-matmul lhsT become single
  contiguous windows (walrus wants 1-free-dim matmul operands).
- Quad-interleaved s-permutation: within each 512-row block, SBUF
  column 128*t + p holds DRAM row 4*p + t. Carried through all
  intermediate tensors and undone in the output store (1KB store runs).
- Landmark segment sums fused into the transpose matmuls:
  rhs = [I128 | ACOL8] (N=136), partials split off to an fp32 strip
  during the PSUM->SBUF copy and summed on DVE. Kills the separate
  per-chunk landmark matmul + its duplicate weight load.
- r3 (kernel_3 row sums) fused into the G matmuls via the 65-column
  [V | mask] lhsT. Kills the per-chunk mask-row matmuls.

All softmaxes skip max-subtraction (logits are ~N(0, 0.125)). Scales
are folded into the ACT exp. Landmarks are kept as segment SUMS (the
/64 is folded into the exp scale).

Newton-Schulz pseudo-inverse is reformulated on N = (1/c) Km^T Km,
which is symmetric, so the whole iteration needs no transposes:
  N_{k+1} = 0.25 N_k Qp(N_k),  Qp(X) = 13I - 15X + 7X^2 - X^3
  R = prod_k 0.25 Qp(N_k)  =>  Vi6 = (1/c) R Km^T
  W = Vi6 @ (diag(1/r3) G) = (1/c) R @ (Km^T G~)
The reference's init scale c = max over ALL (b,h) of colsums of
kernel_2 couples the shards; we compute c exactly on the host (cheap
numpy reduction producing one scalar) and pass 1/c as a tiny input.
"""

import numpy as np

B, H, S, D, L = 8, 12, 4096, 64, 64
NCORES = 8
PER_CORE = (B * H) // NCORES      # 12 slices
NPAIRS = PER_CORE // 2            # 6
NBLK = S // 512                   # 8 blocks of 512 rows
NCHUNK = S // 128                 # 32 chunks (bb, t)
SCALE2 = 0.125                    # (d^-1/4)^2
EXP_SCALE_SL = SCALE2 / 64.0      # for S1, S3 logits (one landmark-sum side)
EXP_SCALE_S2 = SCALE2 / 4096.0    # for S2 logits (two landmark-sum sides)

# fp16 consts column layout
C_I128 = 0        # [128,128] identity (I|ACOL must be adjacent)
C_ACOL = 128      # [128,8] landmark indicator cols (16-row bands)
C_ONES = 136      # [128,1] ones column (r3 reduction lhsT)
C_NCOLS = 137
# fp32 consts
C32_I13 = 0       # [128,64] 13*[I64;I64]
C32_P15 = 64      # [128,128] 15*I
C32_M7 = 192      # [128,128] -7*I
C32_I65 = 320     # [128,65] I65 in rows 0:65
C32_NCOLS = 385

_PROG_CACHE = {}


def _make_consts():
    C = np.zeros((128, C_NCOLS), np.float16)
    I128 = np.eye(128, dtype=np.float16)
    C[:, C_I128:C_I128 + 128] = I128
    for j in range(8):
        C[16 * j:16 * j + 16, C_ACOL + j] = 1.0
    C[:, C_ONES] = 1.0
    C32 = np.zeros((128, C32_NCOLS), np.float32)
    I64 = np.eye(64, dtype=np.float32)
    C32[0:64, C32_I13:C32_I13 + 64] = 13.0 * I64
    C32[64:128, C32_I13:C32_I13 + 64] = 13.0 * I64
    I128f = np.eye(128, dtype=np.float32)
    C32[:, C32_P15:C32_P15 + 128] = 15.0 * I128f
    C32[:, C32_M7:C32_M7 + 128] = -7.0 * I128f
    C32[0:65, C32_I65:C32_I65 + 65] = np.eye(65, dtype=np.float32)
    return C, C32


def _host_global_c(Q, K, mask):
    """Exact global max of kernel_2 column-sums (one fp32 scalar)."""
    scale = np.float32(1.0 / np.sqrt(np.sqrt(D)))
    if mask.min() >= 1.0 and mask.max() <= 1.0:
        Qs = Q
        Ks = K
    else:
        m = mask[:, None, :, None].astype(np.float32)
        Qs = Q * m
        Ks = K * m
    seg = S // L
    Q_l = Qs.reshape(B, H, L, seg, D).mean(axis=-2, dtype=np.float32) * scale
    K_l = Ks.reshape(B, H, L, seg, D).mean(axis=-2, dtype=np.float32) * scale
    s2 = np.einsum('bhld,bhmd->bhlm', Q_l, K_l).astype(np.float32)
    s2 -= s2.max(axis=-1, keepdims=True)
    e = np.exp(s2, dtype=np.float32)
    k2 = e / e.sum(axis=-1, keepdims=True, dtype=np.float32)
    return np.float32(k2.sum(axis=-2, dtype=np.float32).max())


def _build_program(npairs=NPAIRS, debug=False, ones_mask=True):
    import concourse.bacc as bacc
    import concourse.mybir as mybir
    import concourse.tile as tile
    from concourse.bass import ds

    f32 = mybir.dt.float32
    f16 = mybir.dt.float16
    AF = mybir.ActivationFunctionType
    AX = mybir.AxisListType

    per_core = npairs * 2
    nc = bacc.Bacc("TRN2", target_bir_lowering=False, debug=False)
    qd = nc.dram_tensor("q", [npairs, S, 128], f32, kind="ExternalInput").ap()
    kd = nc.dram_tensor("k", [npairs, S, 128], f32, kind="ExternalInput").ap()
    vd = nc.dram_tensor("v", [per_core, S, 68], f32, kind="ExternalInput").ap()
    rcd = nc.dram_tensor("rc", [128, 1], f32, kind="ExternalInput").ap()
    cd = nc.dram_tensor("c", [128, C_NCOLS], f16, kind="ExternalInput").ap()
    cd32 = nc.dram_tensor("c32", [128, C32_NCOLS], f32,
                          kind="ExternalInput").ap()
    xd = nc.dram_tensor("x", [per_core, S, D], f32, kind="ExternalOutput").ap()
    if debug:
        dbg = {
            "dbg_qts": nc.dram_tensor("dbg_qts", [128, 4096], f16,
                                      kind="ExternalOutput").ap(),
            "dbg_pq": nc.dram_tensor("dbg_pq", [128, 256], f32,
                                     kind="ExternalOutput").ap(),
            "dbg_lmq": nc.dram_tensor("dbg_lmq", [128, 64], f32,
                                      kind="ExternalOutput").ap(),
            "dbg_lmk": nc.dram_tensor("dbg_lmk", [128, 64], f32,
                                      kind="ExternalOutput").ap(),
            "dbg_km": nc.dram_tensor("dbg_km", [128, 64], f32,
                                     kind="ExternalOutput").ap(),
            "dbg_gts": nc.dram_tensor("dbg_gts", [128, 128], f32,
                                      kind="ExternalOutput").ap(),
            "dbg_gt": nc.dram_tensor("dbg_gt", [128, 64], f32,
                                     kind="ExternalOutput").ap(),
            "dbg_wbd": nc.dram_tensor("dbg_wbd", [128, 130], f16,
                                      kind="ExternalOutput").ap(),
            "dbg_e1t": nc.dram_tensor("dbg_e1t", [128, 512], f16,
                                      kind="ExternalOutput").ap(),
            "dbg_e3t": nc.dram_tensor("dbg_e3t", [128, 512], f16,
                                      kind="ExternalOutput").ap(),
            "dbg_rst": nc.dram_tensor("dbg_rst", [128, 64], f32,
                                      kind="ExternalOutput").ap(),
        }

    with tile.TileContext(nc) as tc:
        with (
            tc.tile_pool(name="cst", bufs=1) as cpool,
            tc.tile_pool(name="bigT", bufs=2) as bigT,
            tc.tile_pool(name="med", bufs=4) as med,
            tc.tile_pool(name="sml", bufs=2) as sml,
            tc.tile_pool(name="psA", bufs=3, space="PSUM") as psA,
            tc.tile_pool(name="psB", bufs=2, space="PSUM") as psB,
            tc.tile_pool(name="psC", bufs=3, space="PSUM") as psC,
        ):
            cst = cpool.tile([128, C_NCOLS], f16)
            nc.sync.dma_start(out=cst, in_=cd)
            cst32 = cpool.tile([128, C32_NCOLS], f32)
            nc.sync.dma_start(out=cst32, in_=cd32)
            rcb = cpool.tile([128, 1], f32)
            nc.sync.dma_start(out=rcb, in_=rcd)
            IA = cst[:, C_I128:C_I128 + 136]     # [I128 | ACOL8] fp16
            ONES1 = cst[:, C_ONES:C_ONES + 1]    # [128,1] ones fp16
            I13 = cst32[:, C32_I13:C32_I13 + 64]
            P15 = cst32[:, C32_P15:C32_P15 + 128]
            M7 = cst32[:, C32_M7:C32_M7 + 128]
            I65 = cst32[0:65, C32_I65:C32_I65 + 65]

            pending_x = []

            def emit_x(p, a, b, e1t, wbd):
                # xo cols = (h 2, bb 2, t 4, d 64); store runs 1KB both sides
                for u in range(4):  # 1024-row store units (2 blocks each)
                    xo = med.tile([128, 1024], f32, tag="xo",
                                  name=f"xo{p}_{u}")
                    xov = xo.rearrange("p (h bb t d) -> p h bb t d",
                                       h=2, bb=2, t=4)
                    for k in range(4):  # 2 chunks per psum bank
                        ps_x = psC.tile([128, 512], f32, tag="xinv",
                                        name=f"psx{p}_{u}_{k}")
                        for r in range(2):
                            c = 8 * u + 2 * k + r
                            nc.tensor.matmul(
                                ps_x[:, ds(130 * r, 130)],
                                e1t[:, ds(128 * c, 128)], wbd,
                                start=True, stop=True,
                                skip_group_check=True)
                        psxv = ps_x[:, 0:260].rearrange(
                            "p (r h w) -> p r h w", r=2, h=2)
                        rr = sml.tile([128, 4], f32, tag="rr",
                                      name=f"rr{p}_{u}_{k}")
                        rrv = rr.rearrange("p (r h) -> p r h", r=2)
                        nc.vector.reciprocal(
                            rrv, psxv[:, :, :, 64:65]
                            .rearrange("p r h one -> p r (h one)"))
                        bb, t0 = (2 * k) // 4, (2 * k) % 4
                        nc.vector.tensor_mul(
                            xov[:, :, bb, t0:t0 + 2, :],
                            psxv[:, :, :, 0:64]
                            .rearrange("p r h d -> p h r d"),
                            rrv.rearrange("p r h -> p h r")[:, :, :, None]
                            .broadcast_to([128, 2, 2, 64]))
                    for h, sl in ((0, a), (1, b)):
                        nc.sync.dma_start(
                            out=xd[sl, ds(1024 * u, 1024), :]
                            .rearrange("(bb p t) d -> p bb (t d)",
                                       bb=2, p=128),
                            in_=xo.rearrange("p (h c) -> p h c", h=2)[:, h]
                            .rearrange("p (bb c) -> p bb c", bb=2))

            for p in range(npairs):
                a, b = 2 * p, 2 * p + 1

                # ---------- ingest: SWDGE cast fp32 -> fp16 ----------
                # ntq/ntk cols = blk(8) x t(4) x (h d)(128); DRAM runs 2KB.
                # Within block bb, SBUF chunk col 128*t + p <-> row 4*p + t.
                ntq = bigT.tile([128, 4096], f16, tag="ntq", name=f"ntq{p}")
                ntk = bigT.tile([128, 4096], f16, tag="ntk", name=f"ntk{p}")
                for srcd, nt in ((qd, ntq), (kd, ntk)):
                    nc.gpsimd.dma_start(
                        out=nt.rearrange("p (bb c) -> p bb c", bb=NBLK),
                        in_=srcd[p].rearrange("(bb p t) c -> p bb (t c)",
                                              bb=NBLK, p=128))
                # vva/vvb cols = blk(8) x t(4) x (d|mask)(65); DRAM runs ~1KB.
                vva = bigT.tile([128, 2176], f16, tag="vva", name=f"vva{p}")
                vvb = bigT.tile([128, 2176], f16, tag="vvb", name=f"vvb{p}")
                for sl, vv in ((a, vva), (b, vvb)):
                    nc.gpsimd.dma_start(
                        out=vv.rearrange("p (bb c) -> p bb c", bb=NBLK),
                        in_=vd[sl].rearrange("(bb p t) c -> p bb (t c)",
                                             bb=NBLK, p=128))

                # ---------- T phase: fused transpose + landmark sums ----
                # chunk c = 4*bb + t: lhsT = nt[:, 128c:+128] ([s, (h d)]),
                # rhs = [I128 | ACOL8] -> psum [128, 136]: cols 0:128 =
                # chunk.T (qt piece), cols 128:136 = 16-row-band sums
                # (landmark partials for (bb, t, j)). The copy back splits
                # the transpose part (fp16, to qts/kts) from the partials
                # (fp32 strip pq/pk).
                qts = bigT.tile([128, 4096], f16, tag="qts", name=f"qts{p}")
                kts = bigT.tile([128, 4096], f16, tag="kts", name=f"kts{p}")
                pq = sml.tile([128, 256], f32, tag="pq", name=f"pq{p}")
                pk = sml.tile([128, 256], f32, tag="pk", name=f"pk{p}")
                lmq = sml.tile([128, 64], f32, tag="lmq", name=f"lmq{p}")
                lmk = sml.tile([128, 64], f32, tag="lmk", name=f"lmk{p}")
                for ti, (nt, dst, pstrip, lm) in enumerate(
                        ((ntq, qts, pq, lmq), (ntk, kts, pk, lmk))):
                    for g in range(11):  # 3 chunks per psum bank (last: 2)
                        n_in_g = 3 if g < 10 else 2
                        pst = psA.tile([128, 512], f32, tag="bigps",
                                       name=f"pst{p}_{ti}_{g}")
                        for k in range(n_in_g):
                            c = 3 * g + k
                            nc.tensor.matmul(
                                pst[:, ds(136 * k, 136)],
                                nt[:, ds(128 * c, 128)], IA,
                                start=True, stop=True,
                                skip_group_check=True)
                        pstv = pst[:, 0:136 * n_in_g] \
                            .rearrange("p (k w) -> p k w", w=136)
                        tcp = dst[:, ds(384 * g, 128 * n_in_g)] \
                            .rearrange("p (k w) -> p k w", w=128)
                        pcp = pstrip[:, ds(24 * g, 8 * n_in_g)] \
                            .rearrange("p (k w) -> p k w", w=8)
                        if (ti + g) % 2 == 0:
                            nc.vector.tensor_copy(tcp,
                                                  pstv[:, 0:n_in_g, 0:128])
                            nc.scalar.copy(out=pcp,
                                           in_=pstv[:, 0:n_in_g, 128:136])
                        else:
                            nc.scalar.copy(out=tcp,
                                           in_=pstv[:, 0:n_in_g, 0:128])
                            nc.vector.tensor_copy(pcp,
                                                  pstv[:, 0:n_in_g, 128:136])
                    # landmark partials: pstrip[:, 8c : 8c+8] for c =
                    # (bb, t); sum over t on DVE (3 adds). l = 8*bb + j.
                    lv = pstrip.rearrange("p (bb t j) -> p bb t j",
                                          bb=NBLK, t=4)
                    t01 = sml.tile([128, 64], f32, tag="t01",
                                   name=f"t01{p}_{ti}")
                    t01v = t01.rearrange("p (bb j) -> p bb j", bb=NBLK)
                    nc.vector.tensor_add(t01v, lv[:, :, 0, :], lv[:, :, 1, :])
                    t23 = sml.tile([128, 64], f32, tag="t23",
                                   name=f"t23{p}_{ti}")
                    t23v = t23.rearrange("p (bb j) -> p bb j", bb=NBLK)
                    nc.vector.tensor_add(t23v, lv[:, :, 2, :], lv[:, :, 3, :])
                    nc.vector.tensor_add(lm, t01, t23)

                # blockdiag landmark tiles, fp32 (S2/NS path) + fp16 casts
                # (E1/E3 logits path)
                bdq = sml.tile([128, 128], f32, tag="bdq", name=f"bdq{p}")
                bdk = sml.tile([128, 128], f32, tag="bdk", name=f"bdk{p}")
                for bd, lm in ((bdq, lmq), (bdk, lmk)):
                    nc.gpsimd.memset(bd[0:64, 64:128], 0.0)
                    nc.gpsimd.memset(bd[64:128, 0:64], 0.0)
                    nc.vector.tensor_copy(bd[0:64, 0:64], lm[0:64, :])
                    nc.vector.tensor_copy(bd[64:128, 64:128], lm[64:128, :])
                bdq16 = sml.tile([128, 128], f16, tag="bdq16",
                                 name=f"bdq16{p}")
                bdk16 = sml.tile([128, 128], f16, tag="bdk16",
                                 name=f"bdk16{p}")
                nc.vector.tensor_copy(bdq16, bdq)
                nc.scalar.copy(out=bdk16, in_=bdk)

                # ---------- S2 / kernel_2 (fp32) ----------
                ps_s2 = psC.tile([128, 512], f32, tag="xinv", name=f"pss2{p}")
                nc.tensor.matmul(ps_s2[0:64, 0:64], bdq[0:64, 0:64],
                                 bdk[0:64, 0:64], start=True, stop=True,
                                 tile_position=(0, 0))
                nc.tensor.matmul(ps_s2[64:128, 0:64], bdq[64:128, 64:128],
                                 bdk[64:128, 64:128], start=True, stop=True,
                                 tile_position=(64, 64))
                e2 = sml.tile([128, 64], f32, tag="e2", name=f"e2{p}")
                nc.scalar.activation(e2, ps_s2[:, 0:64], AF.Exp,
                                     scale=EXP_SCALE_S2)
                r2 = sml.tile([128, 1], f32, tag="r2", name=f"r2{p}")
                nc.vector.reduce_sum(r2, e2, axis=AX.X)
                nc.vector.reciprocal(r2, r2)
                km = sml.tile([128, 64], f32, tag="km", name=f"km{p}")
                nc.vector.tensor_mul(km, e2, r2.broadcast_to([128, 64]))

                # ---------- N0 = (1/c) Km^T Km (fp32) ----------
                ps_n0 = psC.tile([128, 512], f32, tag="xinv", name=f"psn0{p}")
                nc.tensor.matmul(ps_n0[0:64, 0:64], km[0:64, :], km[0:64, :],
                                 start=True, stop=True, tile_position=(0, 0))
                nc.tensor.matmul(ps_n0[64:128, 0:64], km[64:128, :],
                                 km[64:128, :], start=True, stop=True,
                                 tile_position=(64, 64))
                n_st = sml.tile([128, 64], f32, tag="nst", name=f"n0{p}")
                nc.vector.tensor_mul(n_st, ps_n0[:, 0:64],
                                     rcb.broadcast_to([128, 64]))

                # ---------- Newton-Schulz iteration body (fp32) ----------
                # Emitted INTERLEAVED with the E3/G and E1 groups below so
                # the PE always has independent fp16 work queued between the
                # serially-dependent NS matmul->DVE->matmul round trips
                # (otherwise the PE idles and HAM re-throttles it to 1.2GHz).
                ns = {"n": n_st, "r": None}

                def ns_iter(it):
                    n_st, r_st = ns["n"], ns["r"]
                    ps_sq = psC.tile([128, 512], f32, tag="xinv",
                                     name=f"psq{p}_{it}")
                    nc.tensor.matmul(ps_sq[0:64, 0:64], n_st[0:64, :],
                                     n_st[0:64, :], start=True, stop=True,
                                     tile_position=(0, 0))
                    nc.tensor.matmul(ps_sq[64:128, 0:64], n_st[64:128, :],
                                     n_st[64:128, :], start=True, stop=True,
                                     tile_position=(64, 64))
                    n2 = sml.tile([128, 64], f32, tag="n2", name=f"n2{p}_{it}")
                    nc.vector.tensor_copy(n2, ps_sq[:, 0:64])
                    # Qp' = 15N - 7N^2 + N^3; only N^3 on the PE (quads),
                    # the diagonal-matrix terms are cheaper as fused DVE ops
                    # than as P15/M7 fp32 matmuls (each of those cost a
                    # double LDWEIGHTS + 4-cycle/row matmul).
                    ps_qp = psC.tile([128, 512], f32, tag="xinv",
                                     name=f"psqp{p}_{it}")
                    nc.tensor.matmul(ps_qp[0:64, 0:64], n_st[0:64, :],
                                     n2[0:64, :], start=True, stop=True,
                                     tile_position=(0, 0),
                                     skip_group_check=True)
                    nc.tensor.matmul(ps_qp[64:128, 0:64], n_st[64:128, :],
                                     n2[64:128, :], start=True, stop=True,
                                     tile_position=(64, 64),
                                     skip_group_check=True)
                    # u = 15*N + N^3
                    u_t = sml.tile([128, 64], f32, tag="ut",
                                   name=f"u{p}_{it}")
                    nc.vector.scalar_tensor_tensor(
                        u_t, n_st, 15.0, ps_qp[:, 0:64],
                        op0=mybir.AluOpType.mult, op1=mybir.AluOpType.add)
                    # v = -7*N^2 + u
                    v_t = sml.tile([128, 64], f32, tag="vt",
                                   name=f"v{p}_{it}")
                    nc.vector.scalar_tensor_tensor(
                        v_t, n2, -7.0, u_t,
                        op0=mybir.AluOpType.mult, op1=mybir.AluOpType.add)
                    qp = sml.tile([128, 64], f32, tag="qp", name=f"qp{p}_{it}")
                    nc.vector.tensor_sub(qp, v_t, I13)
                    if it == 0:
                        r_new = sml.tile([128, 64], f32, tag="rst",
                                         name=f"r{p}_{it}")
                        nc.vector.tensor_scalar_mul(r_new, qp, -0.25)
                    else:
                        ps_r = psC.tile([128, 512], f32, tag="xinv",
                                        name=f"psr{p}_{it}")
                        nc.tensor.matmul(ps_r[0:64, 0:64], r_st[0:64, :],
                                         qp[0:64, :], start=True, stop=True,
                                         tile_position=(0, 0))
                        nc.tensor.matmul(ps_r[64:128, 0:64], r_st[64:128, :],
                                         qp[64:128, :], start=True, stop=True,
                                         tile_position=(64, 64))
                        r_new = sml.tile([128, 64], f32, tag="rst",
                                         name=f"r{p}_{it}")
                        nc.vector.tensor_scalar_mul(r_new, ps_r[:, 0:64],
                                                    -0.25)
                    ns["r"] = r_new
                    if it < 5:
                        ps_nn = psC.tile([128, 512], f32, tag="xinv",
                                         name=f"psnn{p}_{it}")
                        nc.tensor.matmul(ps_nn[0:64, 0:64], n_st[0:64, :],
                                         qp[0:64, :], start=True, stop=True,
                                         tile_position=(0, 0))
                        nc.tensor.matmul(ps_nn[64:128, 0:64], n_st[64:128, :],
                                         qp[64:128, :], start=True, stop=True,
                                         tile_position=(64, 64))
                        n_new = sml.tile([128, 64], f32, tag="nst",
                                         name=f"n{p}_{it}")
                        nc.vector.tensor_scalar_mul(n_new, ps_nn[:, 0:64],
                                                    -0.25)
                        ns["n"] = n_new

                # ---------- E3^T and G^T = [V|m]^T E3~^T (fp16 mms) -----
                # One accumulator bank PER SLICE: start=True clears the
                # has_written bits of the whole bank on the written
                # partitions, so two interleaved accumulation streams on the
                # same partitions of one bank lose the first stream's c=0
                # contribution (measured: exactly-missing-chunk-0).
                # Rows 0:64 = G^T, row 64 = r3.
                # previous pair's X phase drops in here: its DVE-heavy
                # normalize/store overlaps this pair's PE-heavy E3/E1 stream
                if pending_x:
                    emit_x(*pending_x.pop(0))

                ps_ga = psB.tile([128, 512], f32, tag="gacc", name=f"psga{p}")
                ps_gb = psB.tile([128, 512], f32, tag="gacc", name=f"psgb{p}")
                e1t = bigT.tile([128, 4096], f16, tag="e1t", name=f"e1t{p}")
                gw = 64 if ones_mask else 65

                def emit_g(g, e3t):
                    # G matmuls for group g (consume e3t[g]); emitted one
                    # group LATE so the in-order PE queue never stalls on
                    # the ACT exp producing e3t (head-of-line blocking was
                    # micro-idling the PE and HAM-throttling it to 1.2GHz).
                    for ci in range(4):
                        c = 4 * g + ci
                        first, last = (c == 0), (c == NCHUNK - 1)
                        vcol = 272 * g + 68 * ci  # (bb=g, t=ci) chunk
                        nc.tensor.matmul(
                            ps_ga[0:gw, 0:64],
                            vva[:, ds(vcol, gw)],
                            e3t[:, ds(128 * ci, 64)],
                            start=first, stop=last,
                            tile_position=(0, 0), skip_group_check=True)
                        nc.tensor.matmul(
                            ps_gb[0:gw, 0:64],
                            vvb[:, ds(vcol, gw)],
                            e3t[:, ds(128 * ci + 64, 64)],
                            start=first, stop=last,
                            tile_position=(0, 0), skip_group_check=True)
                    if ones_mask:
                        # r3 partials for BOTH slices in one N=512 matmul:
                        # out[0, (ci, h, l)] += column sums of e3t; lives on
                        # partition 64 of ps_ga (disjoint from G's 0:64).
                        nc.tensor.matmul(
                            ps_ga[64:65, 0:512], ONES1, e3t,
                            start=(g == 0), stop=(g == 7),
                            tile_position=(0, 64), skip_group_check=True)

                prev_e3t = None
                for g in range(8):
                    ps_e3 = psA.tile([128, 512], f32, tag="bigps",
                                     name=f"pse3{p}_{g}")
                    for ci in range(4):
                        c = 4 * g + ci
                        nc.tensor.matmul(ps_e3[:, ds(128 * ci, 128)],
                                         kts[:, ds(128 * c, 128)], bdq16,
                                         start=True, stop=True,
                                         skip_group_check=True)
                    e3t = med.tile([128, 512], f16, tag="e3t",
                                   name=f"e3t{p}_{g}")
                    nc.scalar.activation(e3t, ps_e3, AF.Exp,
                                         scale=EXP_SCALE_SL)
                    if debug and p == 0 and g == 0:
                        nc.sync.dma_start(out=dbg["dbg_e3t"], in_=e3t)
                    # ---- E1 group g (independent fp16 work) ----
                    ps_s1 = psA.tile([128, 512], f32, tag="bigps",
                                     name=f"pss1{p}_{g}")
                    nc.tensor.matmul(ps_s1, bdk16, qts[:, ds(512 * g, 512)],
                                     start=True, stop=True)
                    nc.scalar.activation(e1t[:, ds(512 * g, 512)], ps_s1,
                                         AF.Exp, scale=EXP_SCALE_SL)
                    # ---- G matmuls of the PREVIOUS group ----
                    if prev_e3t is not None:
                        emit_g(g - 1, prev_e3t)
                    prev_e3t = e3t
                    # ---- NS iteration g, hidden behind the fp16 stream ----
                    if g < 6:
                        ns_iter(g)
                emit_g(7, prev_e3t)
                r_st = ns["r"]
                # transpose G^T (+r3 row) back to [l, (d|r3)], fp32
                gts = sml.tile([128, 128], f32, tag="gts", name=f"gts{p}")
                if ones_mask:
                    nc.vector.tensor_copy(gts[0:64, 0:64], ps_ga[0:64, 0:64])
                    nc.vector.tensor_copy(gts[0:64, 64:128],
                                          ps_gb[0:64, 0:64])
                    # r3 row: copy the partial row out of PSUM first (the
                    # verifier rejects tensor_tensor with both operands in
                    # one PSUM bank), then 3 SBUF adds over ci and a
                    # 1-partition ACT copy into gts row 64.
                    r3tmp = sml.tile([128, 512 + 128], f32, tag="r3t",
                                     name=f"r3t{p}")
                    nc.scalar.copy(out=r3tmp[64:65, 0:512],
                                   in_=ps_ga[64:65, 0:512])
                    r3v = r3tmp[:, 0:512].rearrange(
                        "p (ci c) -> p ci c", ci=4)
                    nc.vector.tensor_add(r3tmp[:, 512:640],
                                         r3v[:, 0, :], r3v[:, 1, :])
                    nc.vector.tensor_add(r3tmp[:, 512:640],
                                         r3tmp[:, 512:640], r3v[:, 2, :])
                    nc.vector.tensor_add(gts[64:65, 0:128],
                                         r3tmp[64:65, 512:640],
                                         r3v[64:65, 3, :])
                else:
                    nc.vector.tensor_copy(gts[0:65, 0:64], ps_ga[0:65, 0:64])
                    nc.vector.tensor_copy(gts[0:65, 64:128],
                                          ps_gb[0:65, 0:64])
                ps_g2 = psC.tile([128, 512], f32, tag="xinv", name=f"psg2{p}")
                nc.tensor.matmul(ps_g2[:, 0:65], gts[0:65, 0:128], I65,
                                 start=True, stop=True)
                r3r = sml.tile([128, 1], f32, tag="r3", name=f"r3{p}")
                nc.vector.reciprocal(r3r, ps_g2[:, 64:65])
                gt = sml.tile([128, 64], f32, tag="gt", name=f"gt{p}")
                nc.vector.tensor_mul(gt, ps_g2[:, 0:64],
                                     r3r.broadcast_to([128, 64]))

                # ---------- W = (1/c) R @ (Km^T G~) (fp32) ----------
                ps_kg = psC.tile([128, 512], f32, tag="xinv", name=f"pskg{p}")
                nc.tensor.matmul(ps_kg[0:64, 0:64], km[0:64, :], gt[0:64, :],
                                 start=True, stop=True, tile_position=(0, 0))
                nc.tensor.matmul(ps_kg[64:128, 0:64], km[64:128, :],
                                 gt[64:128, :], start=True, stop=True,
                                 tile_position=(64, 64))
                kg = sml.tile([128, 64], f32, tag="kg", name=f"kg{p}")
                nc.vector.tensor_copy(kg, ps_kg[:, 0:64])
                ps_w = psC.tile([128, 512], f32, tag="xinv", name=f"psw{p}")
                nc.tensor.matmul(ps_w[0:64, 0:64], r_st[0:64, :], kg[0:64, :],
                                 start=True, stop=True, tile_position=(0, 0))
                nc.tensor.matmul(ps_w[64:128, 0:64], r_st[64:128, :],
                                 kg[64:128, :], start=True, stop=True,
                                 tile_position=(64, 64))
                wbd = sml.tile([128, 130], f16, tag="wbd", name=f"wbd{p}")
                nc.gpsimd.memset(wbd[0:64, 65:130], 0.0)
                nc.gpsimd.memset(wbd[64:128, 0:65], 0.0)
                nc.gpsimd.memset(wbd[0:64, 64:65], 1.0)
                nc.gpsimd.memset(wbd[64:128, 129:130], 1.0)
                nc.vector.tensor_mul(wbd[0:64, 0:64], ps_w[0:64, 0:64],
                                     rcb[0:64, :].broadcast_to([64, 64]))
                nc.vector.tensor_mul(wbd[64:128, 65:129], ps_w[64:128, 0:64],
                                     rcb[64:128, :].broadcast_to([64, 64]))

                if debug and p == 0:
                    nc.sync.dma_start(out=dbg["dbg_qts"], in_=qts)
                    nc.sync.dma_start(out=dbg["dbg_pq"], in_=pq)
                    nc.sync.dma_start(out=dbg["dbg_lmq"], in_=lmq)
                    nc.sync.dma_start(out=dbg["dbg_lmk"], in_=lmk)
                    nc.sync.dma_start(out=dbg["dbg_km"], in_=km)
                    nc.sync.dma_start(out=dbg["dbg_gts"], in_=gts)
                    nc.sync.dma_start(out=dbg["dbg_gt"], in_=gt)
                    nc.sync.dma_start(out=dbg["dbg_wbd"], in_=wbd)
                    nc.sync.dma_start(out=dbg["dbg_e1t"],
                                      in_=e1t[:, 0:512])
                    nc.sync.dma_start(out=dbg["dbg_rst"], in_=r_st)

                # ---------- X = diag(1/r1) E1 W (fp16 mms) ----------
                # Deferred one pair: pair p's X is emitted during pair p+1's
                # T/E3 stream so its DVE-heavy normalize+store overlaps the
                # next pair's PE-heavy phases (and vice versa).
                pending_x.append((p, a, b, e1t, wbd))
            while pending_x:
                emit_x(*pending_x.pop(0))
    return nc


def _get_program(npairs=NPAIRS, debug=False, ones_mask=True):
    key = (npairs, debug, ones_mask)
    if key not in _PROG_CACHE:
        nc = _build_program(npairs, debug, ones_mask)
        if not nc.is_finalized():
            nc.finalize()  # Bacc defers register allocation until finalize
        _PROG_CACHE[key] = nc
    return _PROG_CACHE[key]


def run(inputs, trace=False, trace_kwargs=None, debug=False):
    from concourse import bass_utils
    Q, K, V, mask = (np.asarray(inputs["Q"], np.float32),
                     np.asarray(inputs["K"], np.float32),
                     np.asarray(inputs["V"], np.float32),
                     np.asarray(inputs["mask"], np.float32))
    ones_mask = bool(mask.min() >= 1.0 and mask.max() <= 1.0)
    rc = np.full((128, 1), 1.0 / _host_global_c(Q, K, mask), np.float32)
    consts16, consts32 = _make_consts()

    if ones_mask:
        Qm, Km, Vm = Q, K, V
    else:
        m = mask[:, None, :, None].astype(np.float32)
        Qm, Km, Vm = Q * m, K * m, V * m

    npair_tot = (B * H) // 2
    # pair-interleaved [48, S, 128]: row s = [T_a[s] | T_b[s]]
    Qp = np.ascontiguousarray(
        Qm.reshape(npair_tot, 2, S, D).transpose(0, 2, 1, 3)
        .reshape(npair_tot, S, 128))
    Kp = np.ascontiguousarray(
        Km.reshape(npair_tot, 2, S, D).transpose(0, 2, 1, 3)
        .reshape(npair_tot, S, 128))
    # V with mask appended as column 64, padded to 68 so each DRAM row
    # is 272B (16B-aligned descriptor starts): [96, S, 68]
    Vx = np.zeros((B * H, S, 68), np.float32)
    Vx[:, :, :64] = Vm.reshape(B * H, S, D)
    Vx[:, :, 64] = np.broadcast_to(mask[:, None, :], (B, H, S)) \
        .reshape(B * H, S)

    nc = _get_program(debug=debug, ones_mask=ones_mask)
    in_maps = []
    for c in range(NCORES):
        in_maps.append({
            "q": Qp[c * NPAIRS:(c + 1) * NPAIRS],
            "k": Kp[c * NPAIRS:(c + 1) * NPAIRS],
            "v": Vx[c * PER_CORE:(c + 1) * PER_CORE],
            "rc": rc,
            "c": consts16,
            "c32": consts32,
        })
    res = bass_utils.run_bass_kernel_spmd(
        nc, in_maps, core_ids=list(range(NCORES)), trace=trace,
        **(trace_kwargs or {}))
    X = np.concatenate([r["x"] for r in res.results], axis=0)
    return X.reshape(B, H, S, D), res


def kernel(**inputs):
    X, _ = run(inputs, trace=False)
    return X


if __name__ == "__main__":
    # quick build check
    prog = _get_program()
    print("built ok")


# revision 28
# speedup vs baseline: 1.1693x; 1.1501x over previous
"""NystromAttention Trainium2 Bass kernel (SPMD over 8 NeuronCores).

Sharding: (B,H)=96 slices flattened; core i takes slices [12i, 12i+12),
processed as 6 pairs stacked on the 128-partition dim.

v3 design (vs the 517us fp32 baseline, which was PE-bound at 93.5% with
fp32 4-cycle/row matmuls and 256B DMA descriptors):

- fp16 datapath for every BIG matmul (1 cycle/row on the PE instead of
  fp32's 4, plus fast-weight-load). fp32->fp16 cast happens inside the
  SWDGE ingest DMAs (gpsimd dma_start casts for free).
- The landmark->kernel_2->Newton-Schulz->W chain stays fp32: errors in
  the matrix being pseudo-inverted (and in the R/W product chain) are
  amplified by its conditioning; fp16 there costs 5e-2 rel error
  (measured in numpy emulation), fp32 chain + fp16 big path = 1.3e-4.
  These are all tiny 64x64 matmuls, so the fp32 4-cycle cost is small.
- Host-side DRAM staging: Q/K stored pair-interleaved [48, S, 128]
  (= [Q_a[s] | Q_b[s]] per row) and V stored [96, S, 65] with the mask
  appended as column 64. Ingest DMA runs become 2KB contiguous on the
  DRAM side (375 GB/s class vs 213 GB/s at 256B runs), and the
  [a|b]-fused transpose chunks / [V|mask] G-matmul lhsT become single
  contiguous windows (walrus wants 1-free-dim matmul operands).
- Quad-interleaved s-permutation: within each 512-row block, SBUF
  column 128*t + p holds DRAM row 4*p + t. Carried through all
  intermediate tensors and undone in the output store (1KB store runs).
- Landmark segment sums fused into the transpose matmuls:
  rhs = [I128 | ACOL8] (N=136), partials split off to an fp32 strip
  during the PSUM->SBUF copy and summed on DVE. Kills the separate
  per-chunk landmark matmul + its duplicate weight load.
- r3 (kernel_3 row sums) fused into the G matmuls via the 65-column
  [V | mask] lhsT. Kills the per-chunk mask-row matmuls.

All softmaxes skip max-subtraction (logits are ~N(0, 0.125)). Scales
are folded into the ACT exp. Landmarks are kept as segment SUMS (the
/64 is folded into the exp scale).

Newton-Schulz pseudo-inverse is reformulated on N = (1/c) Km^T Km,
which is symmetric, so the whole iteration needs no transposes:
  N_{k+1} = 0.25 N_k Qp(N_k),  Qp(X) = 13I - 15X + 7X^2 - X^3
  R = prod_k 0.25 Qp(N_k)  =>  Vi6 = (1/c) R Km^T
  W = Vi6 @ (diag(1/r3) G) = (1/c) R @ (Km^T G~)
The reference's init scale c = max over ALL (b,h) of colsums of
kernel_2 couples the shards; we compute c exactly on the host (cheap
numpy reduction producing one scalar) and pass 1/c as a tiny input.
"""

import numpy as np

B, H, S, D, L = 8, 12, 4096, 64, 64
NCORES = 8
PER_CORE = (B * H) // NCORES      # 12 slices
NPAIRS = PER_CORE // 2            # 6
NBLK = S // 512                   # 8 blocks of 512 rows
NCHUNK = S // 128                 # 32 chunks (bb, t)
SCALE2 = 0.125                    # (d^-1/4)^2
EXP_SCALE_SL = SCALE2 / 64.0      # for S1, S3 logits (one landmark-sum side)
EXP_SCALE_S2 = SCALE2 / 4096.0    # for S2 logits (two landmark-sum sides)

# fp16 consts column layout
C_I128 = 0        # [128,128] identity (I|ACOL must be adjacent)
C_ACOL = 128      # [128,8] landmark indicator cols (16-row bands)
C_ONES = 136      # [128,1] ones column (r3 reduction lhsT)
C_NCOLS = 137
# fp32 consts
C32_I13 = 0       # [128,64] 13*[I64;I64]
C32_P15 = 64      # [128,128] 15*I
C32_M7 = 192      # [128,128] -7*I
C32_I65 = 320     # [128,65] I65 in rows 0:65
C32_NCOLS = 385

_PROG_CACHE = {}


def _make_consts():
    C = np.zeros((128, C_NCOLS), np.float16)
    I128 = np.eye(128, dtype=np.float16)
    C[:, C_I128:C_I128 + 128] = I128
    for j in range(8):
        C[16 * j:16 * j + 16, C_ACOL + j] = 1.0
    C[:, C_ONES] = 1.0
    C32 = np.zeros((128, C32_NCOLS), np.float32)
    I64 = np.eye(64, dtype=np.float32)
    C32[0:64, C32_I13:C32_I13 + 64] = 13.0 * I64
    C32[64:128, C32_I13:C32_I13 + 64] = 13.0 * I64
    I128f = np.eye(128, dtype=np.float32)
    C32[:, C32_P15:C32_P15 + 128] = 15.0 * I128f
    C32[:, C32_M7:C32_M7 + 128] = -7.0 * I128f
    C32[0:65, C32_I65:C32_I65 + 65] = np.eye(65, dtype=np.float32)
    return C, C32


def _host_global_c(Q, K, mask):
    """Exact global max of kernel_2 column-sums (one fp32 scalar)."""
    scale = np.float32(1.0 / np.sqrt(np.sqrt(D)))
    if mask.min() >= 1.0 and mask.max() <= 1.0:
        Qs = Q
        Ks = K
    else:
        m = mask[:, None, :, None].astype(np.float32)
        Qs = Q * m
        Ks = K * m
    seg = S // L
    Q_l = Qs.reshape(B, H, L, seg, D).mean(axis=-2, dtype=np.float32) * scale
    K_l = Ks.reshape(B, H, L, seg, D).mean(axis=-2, dtype=np.float32) * scale
    s2 = np.einsum('bhld,bhmd->bhlm', Q_l, K_l).astype(np.float32)
    s2 -= s2.max(axis=-1, keepdims=True)
    e = np.exp(s2, dtype=np.float32)
    k2 = e / e.sum(axis=-1, keepdims=True, dtype=np.float32)
    return np.float32(k2.sum(axis=-2, dtype=np.float32).max())


def _build_program(npairs=NPAIRS, debug=False, ones_mask=True):
    import concourse.bacc as bacc
    import concourse.mybir as mybir
    import concourse.tile as tile
    from concourse.bass import ds

    f32 = mybir.dt.float32
    f16 = mybir.dt.float16
    AF = mybir.ActivationFunctionType
    AX = mybir.AxisListType
    OP = mybir.AluOpType

    per_core = npairs * 2
    nc = bacc.Bacc("TRN2", target_bir_lowering=False, debug=False)
    qd = nc.dram_tensor("q", [npairs, S, 128], f32, kind="ExternalInput").ap()
    kd = nc.dram_tensor("k", [npairs, S, 128], f32, kind="ExternalInput").ap()
    vd = nc.dram_tensor("v", [per_core, S, 68], f32, kind="ExternalInput").ap()
    rcd = nc.dram_tensor("rc", [128, 1], f32, kind="ExternalInput").ap()
    cd = nc.dram_tensor("c", [128, C_NCOLS], f16, kind="ExternalInput").ap()
    cd32 = nc.dram_tensor("c32", [128, C32_NCOLS], f32,
                          kind="ExternalInput").ap()
    xd = nc.dram_tensor("x", [per_core, S, D], f32, kind="ExternalOutput").ap()

    with tile.TileContext(nc) as tc:
        with (
            tc.tile_pool(name="cst", bufs=1) as cpool,
            tc.tile_pool(name="bigT", bufs=2) as bigT,
            tc.tile_pool(name="med", bufs=4) as med,
            tc.tile_pool(name="sml", bufs=2) as sml,
            tc.tile_pool(name="psA", bufs=3, space="PSUM") as psA,
            tc.tile_pool(name="psB", bufs=2, space="PSUM") as psB,
            tc.tile_pool(name="psC", bufs=3, space="PSUM") as psC,
        ):
            cst = cpool.tile([128, C_NCOLS], f16)
            nc.sync.dma_start(out=cst, in_=cd)
            cst32 = cpool.tile([128, C32_NCOLS], f32)
            nc.sync.dma_start(out=cst32, in_=cd32)
            rcb = cpool.tile([128, 1], f32)
            nc.sync.dma_start(out=rcb, in_=rcd)
            IA = cst[:, C_I128:C_I128 + 136]     # [I128 | ACOL8] fp16
            I13 = cst32[:, C32_I13:C32_I13 + 64]
            I65 = cst32[0:65, C32_I65:C32_I65 + 65]

            # ============================================================
            # The PE executes its queue IN ORDER: any matmul that waits on
            # a DVE/ACT round trip blocks every matmul behind it, the PE
            # micro-idles, and HAM re-throttles the array to 1.2 GHz.  So
            # everything serial (Newton-Schulz, S2->km, the W chain, the
            # G/X finalizes) is chopped into small parts and emitted spread
            # out BETWEEN the dense fp16 matmul groups, and the per-pair
            # phases are software-pipelined across pairs:
            #   pair p emission: ingest(p), T(p), [W-chain(p-1)], S2(p),
            #                    [X(p-1)], E3G/E1(p) x NS-parts(p)
            # ============================================================

            def emit_ingest(p, st):
                a, b = 2 * p, 2 * p + 1
                st["ntq"] = bigT.tile([128, 4096], f16, tag="ntq",
                                      name=f"ntq{p}")
                st["ntk"] = bigT.tile([128, 4096], f16, tag="ntk",
                                      name=f"ntk{p}")
                for srcd, nt in ((qd, st["ntq"]), (kd, st["ntk"])):
                    nc.gpsimd.dma_start(
                        out=nt.rearrange("p (bb c) -> p bb c", bb=NBLK),
                        in_=srcd[p].rearrange("(bb p t) c -> p bb (t c)",
                                              bb=NBLK, p=128))
                st["vva"] = bigT.tile([128, 2176], f16, tag="vva",
                                      name=f"vva{p}")
                st["vvb"] = bigT.tile([128, 2176], f16, tag="vvb",
                                      name=f"vvb{p}")
                for sl, vv in ((a, st["vva"]), (b, st["vvb"])):
                    nc.gpsimd.dma_start(
                        out=vv.rearrange("p (bb c) -> p bb c", bb=NBLK),
                        in_=vd[sl].rearrange("(bb p t) c -> p bb (t c)",
                                             bb=NBLK, p=128))

            def emit_T(p, st):
                # fused transpose + landmark partial sums; chunk c = 4bb+t
                st["qts"] = bigT.tile([128, 4096], f16, tag="qts",
                                      name=f"qts{p}")
                st["kts"] = bigT.tile([128, 4096], f16, tag="kts",
                                      name=f"kts{p}")
                st["pq"] = sml.tile([128, 256], f32, tag="pq", name=f"pq{p}")
                st["pk"] = sml.tile([128, 256], f32, tag="pk", name=f"pk{p}")
                for ti, (nt, dst, pstrip) in enumerate(
                        ((st["ntq"], st["qts"], st["pq"]),
                         (st["ntk"], st["kts"], st["pk"]))):
                    for g in range(11):  # 3 chunks per psum bank (last: 2)
                        n_in_g = 3 if g < 10 else 2
                        pst = psA.tile([128, 512], f32, tag="bigps",
                                       name=f"pst{p}_{ti}_{g}")
                        for k in range(n_in_g):
                            c = 3 * g + k
                            nc.tensor.matmul(
                                pst[:, ds(136 * k, 136)],
                                nt[:, ds(128 * c, 128)], IA,
                                start=True, stop=True,
                                skip_group_check=True)
                        pstv = pst[:, 0:136 * n_in_g] \
                            .rearrange("p (k w) -> p k w", w=136)
                        tcp = dst[:, ds(384 * g, 128 * n_in_g)] \
                            .rearrange("p (k w) -> p k w", w=128)
                        pcp = pstrip[:, ds(24 * g, 8 * n_in_g)] \
                            .rearrange("p (k w) -> p k w", w=8)
                        if (ti + g) % 2 == 0:
                            nc.vector.tensor_copy(tcp,
                                                  pstv[:, 0:n_in_g, 0:128])
                            nc.scalar.copy(out=pcp,
                                           in_=pstv[:, 0:n_in_g, 128:136])
                        else:
                            nc.scalar.copy(out=tcp,
                                           in_=pstv[:, 0:n_in_g, 0:128])
                            nc.vector.tensor_copy(pcp,
                                                  pstv[:, 0:n_in_g, 128:136])

            def emit_bd(p, st):
                # landmark t-sums (fp32) + blockdiag tiles
                st["lmq"] = sml.tile([128, 64], f32, tag="lmq",
                                     name=f"lmq{p}")
                st["lmk"] = sml.tile([128, 64], f32, tag="lmk",
                                     name=f"lmk{p}")
                for ti, (pstrip, lm) in enumerate(
                        ((st["pq"], st["lmq"]), (st["pk"], st["lmk"]))):
                    lv = pstrip.rearrange("p (bb t j) -> p bb t j",
                                          bb=NBLK, t=4)
                    t01 = sml.tile([128, 64], f32, tag="t01",
                                   name=f"t01{p}_{ti}")
                    t01v = t01.rearrange("p (bb j) -> p bb j", bb=NBLK)
                    nc.vector.tensor_add(t01v, lv[:, :, 0, :], lv[:, :, 1, :])
                    t23 = sml.tile([128, 64], f32, tag="t23",
                                   name=f"t23{p}_{ti}")
                    t23v = t23.rearrange("p (bb j) -> p bb j", bb=NBLK)
                    nc.vector.tensor_add(t23v, lv[:, :, 2, :], lv[:, :, 3, :])
                    nc.vector.tensor_add(lm, t01, t23)
                bdq = sml.tile([128, 128], f32, tag="bdq", name=f"bdq{p}")
                bdk = sml.tile([128, 128], f32, tag="bdk", name=f"bdk{p}")
                for bd, lm in ((bdq, st["lmq"]), (bdk, st["lmk"])):
                    nc.gpsimd.memset(bd[0:64, 64:128], 0.0)
                    nc.gpsimd.memset(bd[64:128, 0:64], 0.0)
                    nc.vector.tensor_copy(bd[0:64, 0:64], lm[0:64, :])
                    nc.vector.tensor_copy(bd[64:128, 64:128], lm[64:128, :])
                st["bdq"], st["bdk"] = bdq, bdk
                st["bdq16"] = sml.tile([128, 128], f16, tag="bdq16",
                                       name=f"bdq16{p}")
                st["bdk16"] = sml.tile([128, 128], f16, tag="bdk16",
                                       name=f"bdk16{p}")
                nc.vector.tensor_copy(st["bdq16"], bdq)
                nc.scalar.copy(out=st["bdk16"], in_=bdk)

            def emit_s2(p, st):
                ps_s2 = psC.tile([128, 512], f32, tag="xinv",
                                 name=f"pss2{p}")
                nc.tensor.matmul(ps_s2[0:64, 0:64], st["bdq"][0:64, 0:64],
                                 st["bdk"][0:64, 0:64], start=True, stop=True,
                                 tile_position=(0, 0))
                nc.tensor.matmul(ps_s2[64:128, 0:64],
                                 st["bdq"][64:128, 64:128],
                                 st["bdk"][64:128, 64:128],
                                 start=True, stop=True,
                                 tile_position=(64, 64))
                st["ps_s2"] = ps_s2

            # --- Newton-Schulz, chopped into queue-friendly parts -------
            def ns_km(p, st):
                # exp/softmax of S2 -> km, then N0 matmuls + scale
                e2 = sml.tile([128, 64], f32, tag="e2", name=f"e2{p}")
                nc.scalar.activation(e2, st["ps_s2"][:, 0:64], AF.Exp,
                                     scale=EXP_SCALE_S2)
                r2 = sml.tile([128, 1], f32, tag="r2", name=f"r2{p}")
                nc.vector.reduce_sum(r2, e2, axis=AX.X)
                nc.vector.reciprocal(r2, r2)
                km = sml.tile([128, 64], f32, tag="km", name=f"km{p}")
                nc.vector.tensor_mul(km, e2, r2.broadcast_to([128, 64]))
                st["km"] = km
                ps_n0 = psC.tile([128, 512], f32, tag="xinv",
                                 name=f"psn0{p}")
                nc.tensor.matmul(ps_n0[0:64, 0:64], km[0:64, :], km[0:64, :],
                                 start=True, stop=True, tile_position=(0, 0))
                nc.tensor.matmul(ps_n0[64:128, 0:64], km[64:128, :],
                                 km[64:128, :], start=True, stop=True,
                                 tile_position=(64, 64))
                n_st = sml.tile([128, 64], f32, tag="nst", name=f"n0{p}")
                nc.vector.tensor_mul(n_st, ps_n0[:, 0:64],
                                     rcb.broadcast_to([128, 64]))
                st["ns_n"] = n_st

            def ns_sq(p, st, it):
                n_st = st["ns_n"]
                ps_sq = psC.tile([128, 512], f32, tag="xinv",
                                 name=f"psq{p}_{it}")
                nc.tensor.matmul(ps_sq[0:64, 0:64], n_st[0:64, :],
                                 n_st[0:64, :], start=True, stop=True,
                                 tile_position=(0, 0))
                nc.tensor.matmul(ps_sq[64:128, 0:64], n_st[64:128, :],
                                 n_st[64:128, :], start=True, stop=True,
                                 tile_position=(64, 64))
                n2 = sml.tile([128, 64], f32, tag="n2", name=f"n2{p}_{it}")
                nc.vector.tensor_copy(n2, ps_sq[:, 0:64])
                st["ns_n2"] = n2

            def ns_n3(p, st, it):
                n_st, n2 = st["ns_n"], st["ns_n2"]
                ps_qp = psC.tile([128, 512], f32, tag="xinv",
                                 name=f"psqp{p}_{it}")
                nc.tensor.matmul(ps_qp[0:64, 0:64], n_st[0:64, :],
                                 n2[0:64, :], start=True, stop=True,
                                 tile_position=(0, 0),
                                 skip_group_check=True)
                nc.tensor.matmul(ps_qp[64:128, 0:64], n_st[64:128, :],
                                 n2[64:128, :], start=True, stop=True,
                                 tile_position=(64, 64),
                                 skip_group_check=True)
                # u = 15*N + N^3 ; v = -7*N^2 + u ; qp = v - 13I
                u_t = sml.tile([128, 64], f32, tag="ut", name=f"u{p}_{it}")
                nc.vector.scalar_tensor_tensor(
                    u_t, n_st, 15.0, ps_qp[:, 0:64],
                    op0=OP.mult, op1=OP.add)
                v_t = sml.tile([128, 64], f32, tag="vt", name=f"v{p}_{it}")
                nc.vector.scalar_tensor_tensor(
                    v_t, n2, -7.0, u_t, op0=OP.mult, op1=OP.add)
                qp = sml.tile([128, 64], f32, tag="qp", name=f"qp{p}_{it}")
                nc.vector.tensor_sub(qp, v_t, I13)
                st["ns_qp"] = qp

            def ns_rn(p, st, it):
                n_st, qp, r_st = st["ns_n"], st["ns_qp"], st["ns_r"]
                if it == 0:
                    r_new = sml.tile([128, 64], f32, tag="rst",
                                     name=f"r{p}_{it}")
                    nc.vector.tensor_scalar_mul(r_new, qp, -0.25)
                else:
                    ps_r = psC.tile([128, 512], f32, tag="xinv",
                                    name=f"psr{p}_{it}")
                    nc.tensor.matmul(ps_r[0:64, 0:64], r_st[0:64, :],
                                     qp[0:64, :], start=True, stop=True,
                                     tile_position=(0, 0))
                    nc.tensor.matmul(ps_r[64:128, 0:64], r_st[64:128, :],
                                     qp[64:128, :], start=True, stop=True,
                                     tile_position=(64, 64))
                    r_new = sml.tile([128, 64], f32, tag="rst",
                                     name=f"r{p}_{it}")
                    nc.vector.tensor_scalar_mul(r_new, ps_r[:, 0:64], -0.25)
                st["ns_r"] = r_new
                if it < 5:
                    ps_nn = psC.tile([128, 512], f32, tag="xinv",
                                     name=f"psnn{p}_{it}")
                    nc.tensor.matmul(ps_nn[0:64, 0:64], n_st[0:64, :],
                                     qp[0:64, :], start=True, stop=True,
                                     tile_position=(0, 0))
                    nc.tensor.matmul(ps_nn[64:128, 0:64], n_st[64:128, :],
                                     qp[64:128, :], start=True, stop=True,
                                     tile_position=(64, 64))
                    n_new = sml.tile([128, 64], f32, tag="nst",
                                     name=f"n{p}_{it}")
                    nc.vector.tensor_scalar_mul(n_new, ps_nn[:, 0:64], -0.25)
                    st["ns_n"] = n_new

            def emit_wchain(p, st):
                # G finalize: transpose G^T (+r3 row) back to [l, (d|r3)]
                gts = sml.tile([128, 128], f32, tag="gts", name=f"gts{p}")
                nc.vector.tensor_copy(gts[0:65, 0:64],
                                      st["ps_ga"][0:65, 0:64])
                nc.vector.tensor_copy(gts[0:65, 64:128],
                                      st["ps_gb"][0:65, 0:64])
                ps_g2 = psC.tile([128, 512], f32, tag="xinv",
                                 name=f"psg2{p}")
                nc.tensor.matmul(ps_g2[:, 0:65], gts[0:65, 0:128], I65,
                                 start=True, stop=True)
                r3r = sml.tile([128, 1], f32, tag="r3", name=f"r3{p}")
                nc.vector.reciprocal(r3r, ps_g2[:, 64:65])
                gt = sml.tile([128, 64], f32, tag="gt", name=f"gt{p}")
                nc.vector.tensor_mul(gt, ps_g2[:, 0:64],
                                     r3r.broadcast_to([128, 64]))
                st["gt"] = gt

            def emit_kg(p, st):
                km, gt = st["km"], st["gt"]
                ps_kg = psC.tile([128, 512], f32, tag="xinv",
                                 name=f"pskg{p}")
                nc.tensor.matmul(ps_kg[0:64, 0:64], km[0:64, :], gt[0:64, :],
                                 start=True, stop=True, tile_position=(0, 0))
                nc.tensor.matmul(ps_kg[64:128, 0:64], km[64:128, :],
                                 gt[64:128, :], start=True, stop=True,
                                 tile_position=(64, 64))
                kg = sml.tile([128, 64], f32, tag="kg", name=f"kg{p}")
                nc.vector.tensor_copy(kg, ps_kg[:, 0:64])
                st["kg"] = kg

            def emit_w(p, st):
                kg, r_st = st["kg"], st["ns_r"]
                ps_w = psC.tile([128, 512], f32, tag="xinv", name=f"psw{p}")
                nc.tensor.matmul(ps_w[0:64, 0:64], r_st[0:64, :], kg[0:64, :],
                                 start=True, stop=True, tile_position=(0, 0))
                nc.tensor.matmul(ps_w[64:128, 0:64], r_st[64:128, :],
                                 kg[64:128, :], start=True, stop=True,
                                 tile_position=(64, 64))
                wbd = sml.tile([128, 130], f16, tag="wbd", name=f"wbd{p}")
                nc.gpsimd.memset(wbd[0:64, 65:130], 0.0)
                nc.gpsimd.memset(wbd[64:128, 0:65], 0.0)
                nc.gpsimd.memset(wbd[0:64, 64:65], 1.0)
                nc.gpsimd.memset(wbd[64:128, 129:130], 1.0)
                nc.vector.tensor_mul(wbd[0:64, 0:64], ps_w[0:64, 0:64],
                                     rcb[0:64, :].broadcast_to([64, 64]))
                nc.vector.tensor_mul(wbd[64:128, 65:129], ps_w[64:128, 0:64],
                                     rcb[64:128, :].broadcast_to([64, 64]))
                st["wbd"] = wbd

            def emit_x(p, st):
                a, b = 2 * p, 2 * p + 1
                e1t, wbd = st["e1t"], st["wbd"]
                # xo cols = (h 2, bb 2, t 4, d 64); store runs 1KB both sides
                for u in range(4):  # 1024-row store units (2 blocks each)
                    xo = med.tile([128, 1024], f32, tag="xo",
                                  name=f"xo{p}_{u}")
                    xov = xo.rearrange("p (h bb t d) -> p h bb t d",
                                       h=2, bb=2, t=4)
                    for k in range(4):  # 2 chunks per psum bank
                        ps_x = psC.tile([128, 512], f32, tag="xinv",
                                        name=f"psx{p}_{u}_{k}")
                        for r in range(2):
                            c = 8 * u + 2 * k + r
                            nc.tensor.matmul(
                                ps_x[:, ds(130 * r, 130)],
                                e1t[:, ds(128 * c, 128)], wbd,
                                start=True, stop=True,
                                skip_group_check=True)
                        psxv = ps_x[:, 0:260].rearrange(
                            "p (r h w) -> p r h w", r=2, h=2)
                        rr = sml.tile([128, 4], f32, tag="rr",
                                      name=f"rr{p}_{u}_{k}")
                        rrv = rr.rearrange("p (r h) -> p r h", r=2)
                        nc.vector.reciprocal(
                            rrv, psxv[:, :, :, 64:65]
                            .rearrange("p r h one -> p r (h one)"))
                        bb, t0 = (2 * k) // 4, (2 * k) % 4
                        nc.vector.tensor_mul(
                            xov[:, :, bb, t0:t0 + 2, :],
                            psxv[:, :, :, 0:64]
                            .rearrange("p r h d -> p h r d"),
                            rrv.rearrange("p r h -> p h r")[:, :, :, None]
                            .broadcast_to([128, 2, 2, 64]))
                    for h, sl in ((0, a), (1, b)):
                        nc.sync.dma_start(
                            out=xd[sl, ds(1024 * u, 1024), :]
                            .rearrange("(bb p t) d -> p bb (t d)",
                                       bb=2, p=128),
                            in_=xo.rearrange("p (h c) -> p h c", h=2)[:, h]
                            .rearrange("p (bb c) -> p bb c", bb=2))

            def emit_e3g_loop(p, st):
                # dense fp16 stream: E3 groups + E1 groups + (one group
                # late) G matmuls, with the serial NS parts dropped in
                # between so their DVE round trips hide under fp16 MMs.
                st["ps_ga"] = psB.tile([128, 512], f32, tag="gacc",
                                       name=f"psga{p}")
                st["ps_gb"] = psB.tile([128, 512], f32, tag="gacc",
                                       name=f"psgb{p}")
                st["e1t"] = bigT.tile([128, 4096], f16, tag="e1t",
                                      name=f"e1t{p}")
                kts, qts = st["kts"], st["qts"]
                vva, vvb = st["vva"], st["vvb"]
                bdq16, bdk16 = st["bdq16"], st["bdk16"]
                e1t = st["e1t"]

                parts = [lambda: ns_km(p, st)]
                for it in range(6):
                    parts.append(lambda it=it: ns_sq(p, st, it))
                    parts.append(lambda it=it: ns_n3(p, st, it))
                    parts.append(lambda it=it: ns_rn(p, st, it))

                def pop_part():
                    if parts:
                        parts.pop(0)()

                def emit_g(g, e3t):
                    gw = 64 if ones_mask else 65
                    for ci in range(4):
                        c = 4 * g + ci
                        first, last = (c == 0), (c == NCHUNK - 1)
                        vcol = 272 * g + 68 * ci  # (bb=g, t=ci) chunk
                        nc.tensor.matmul(
                            st["ps_ga"][0:65, 0:64],
                            vva[:, ds(vcol, 65)],
                            e3t[:, ds(128 * ci, 64)],
                            start=first, stop=last,
                            tile_position=(0, 0), skip_group_check=True)
                        nc.tensor.matmul(
                            st["ps_gb"][0:65, 0:64],
                            vvb[:, ds(vcol, 65)],
                            e3t[:, ds(128 * ci + 64, 64)],
                            start=first, stop=last,
                            tile_position=(0, 0), skip_group_check=True)

                prev_e3t = None
                for g in range(8):
                    ps_e3 = psA.tile([128, 512], f32, tag="bigps",
                                     name=f"pse3{p}_{g}")
                    for ci in range(4):
                        c = 4 * g + ci
                        nc.tensor.matmul(ps_e3[:, ds(128 * ci, 128)],
                                         kts[:, ds(128 * c, 128)], bdq16,
                                         start=True, stop=True,
                                         skip_group_check=True)
                    e3t = med.tile([128, 512], f16, tag="e3t",
                                   name=f"e3t{p}_{g}")
                    nc.scalar.activation(e3t, ps_e3, AF.Exp,
                                         scale=EXP_SCALE_SL)
                    pop_part()
                    ps_s1 = psA.tile([128, 512], f32, tag="bigps",
                                     name=f"pss1{p}_{g}")
                    nc.tensor.matmul(ps_s1, bdk16, qts[:, ds(512 * g, 512)],
                                     start=True, stop=True)
                    nc.scalar.activation(e1t[:, ds(512 * g, 512)], ps_s1,
                                         AF.Exp, scale=EXP_SCALE_SL)
                    if prev_e3t is not None:
                        emit_g(g - 1, prev_e3t)
                    prev_e3t = e3t
                    pop_part()
                    if g in (3, 5, 7):
                        pop_part()
                emit_g(7, prev_e3t)
                while parts:
                    pop_part()

            # ---------------- pipelined pair loop -----------------------
            prev = None
            for p in range(npairs):
                st = {"ns_r": None}
                emit_ingest(p, st)
                emit_T(p, st)
                emit_bd(p, st)
                if prev is not None:
                    emit_wchain(prev["p"], prev)
                    emit_kg(prev["p"], prev)
                emit_s2(p, st)
                if prev is not None:
                    emit_w(prev["p"], prev)
                    emit_x(prev["p"], prev)
                emit_e3g_loop(p, st)
                st["p"] = p
                prev = st
            emit_wchain(prev["p"], prev)
            emit_kg(prev["p"], prev)
            emit_w(prev["p"], prev)
            emit_x(prev["p"], prev)
    return nc


def _get_program(npairs=NPAIRS, debug=False, ones_mask=True):
    key = (npairs, debug, ones_mask)
    if key not in _PROG_CACHE:
        nc = _build_program(npairs, debug, ones_mask)
        if not nc.is_finalized():
            nc.finalize()  # Bacc defers register allocation until finalize
        _PROG_CACHE[key] = nc
    return _PROG_CACHE[key]


def run(inputs, trace=False, trace_kwargs=None, debug=False):
    from concourse import bass_utils
    Q, K, V, mask = (np.asarray(inputs["Q"], np.float32),
                     np.asarray(inputs["K"], np.float32),
                     np.asarray(inputs["V"], np.float32),
                     np.asarray(inputs["mask"], np.float32))
    ones_mask = bool(mask.min() >= 1.0 and mask.max() <= 1.0)
    rc = np.full((128, 1), 1.0 / _host_global_c(Q, K, mask), np.float32)
    consts16, consts32 = _make_consts()

    if ones_mask:
        Qm, Km, Vm = Q, K, V
    else:
        m = mask[:, None, :, None].astype(np.float32)
        Qm, Km, Vm = Q * m, K * m, V * m

    npair_tot = (B * H) // 2
    # pair-interleaved [48, S, 128]: row s = [T_a[s] | T_b[s]]
    Qp = np.ascontiguousarray(
        Qm.reshape(npair_tot, 2, S, D).transpose(0, 2, 1, 3)
        .reshape(npair_tot, S, 128))
    Kp = np.ascontiguousarray(
        Km.reshape(npair_tot, 2, S, D).transpose(0, 2, 1, 3)
        .reshape(npair_tot, S, 128))
    # V with mask appended as column 64, padded to 68 so each DRAM row
    # is 272B (16B-aligned descriptor starts): [96, S, 68]
    Vx = np.zeros((B * H, S, 68), np.float32)
    Vx[:, :, :64] = Vm.reshape(B * H, S, D)
    Vx[:, :, 64] = np.broadcast_to(mask[:, None, :], (B, H, S)) \
        .reshape(B * H, S)

    nc = _get_program(debug=debug, ones_mask=ones_mask)
    in_maps = []
    for c in range(NCORES):
        in_maps.append({
            "q": Qp[c * NPAIRS:(c + 1) * NPAIRS],
            "k": Kp[c * NPAIRS:(c + 1) * NPAIRS],
            "v": Vx[c * PER_CORE:(c + 1) * PER_CORE],
            "rc": rc,
            "c": consts16,
            "c32": consts32,
        })
    res = bass_utils.run_bass_kernel_spmd(
        nc, in_maps, core_ids=list(range(NCORES)), trace=trace,
        **(trace_kwargs or {}))
    X = np.concatenate([r["x"] for r in res.results], axis=0)
    return X.reshape(B, H, S, D), res


def kernel(**inputs):
    X, _ = run(inputs, trace=False)
    return X


if __name__ == "__main__":
    # quick build check
    prog = _get_program()
    print("built ok")


# revision 29
# speedup vs baseline: 1.2690x; 1.0853x over previous
"""NystromAttention Trainium2 Bass kernel (SPMD over 8 NeuronCores).

Sharding: (B,H)=96 slices flattened; core i takes slices [12i, 12i+12),
processed as 6 pairs stacked on the 128-partition dim.

v3 design (vs the 517us fp32 baseline, which was PE-bound at 93.5% with
fp32 4-cycle/row matmuls and 256B DMA descriptors):

- fp16 datapath for every BIG matmul (1 cycle/row on the PE instead of
  fp32's 4, plus fast-weight-load). fp32->fp16 cast happens inside the
  SWDGE ingest DMAs (gpsimd dma_start casts for free).
- The landmark->kernel_2->Newton-Schulz->W chain stays fp32: errors in
  the matrix being pseudo-inverted (and in the R/W product chain) are
  amplified by its conditioning; fp16 there costs 5e-2 rel error
  (measured in numpy emulation), fp32 chain + fp16 big path = 1.3e-4.
  These are all tiny 64x64 matmuls, so the fp32 4-cycle cost is small.
- Host-side DRAM staging: Q/K stored pair-interleaved [48, S, 128]
  (= [Q_a[s] | Q_b[s]] per row) and V stored [96, S, 65] with the mask
  appended as column 64. Ingest DMA runs become 2KB contiguous on the
  DRAM side (375 GB/s class vs 213 GB/s at 256B runs), and the
  [a|b]-fused transpose chunks / [V|mask] G-matmul lhsT become single
  contiguous windows (walrus wants 1-free-dim matmul operands).
- Quad-interleaved s-permutation: within each 512-row block, SBUF
  column 128*t + p holds DRAM row 4*p + t. Carried through all
  intermediate tensors and undone in the output store (1KB store runs).
- Landmark segment sums fused into the transpose matmuls:
  rhs = [I128 | ACOL8] (N=136), partials split off to an fp32 strip
  during the PSUM->SBUF copy and summed on DVE. Kills the separate
  per-chunk landmark matmul + its duplicate weight load.
- r3 (kernel_3 row sums) fused into the G matmuls via the 65-column
  [V | mask] lhsT. Kills the per-chunk mask-row matmuls.

All softmaxes skip max-subtraction (logits are ~N(0, 0.125)). Scales
are folded into the ACT exp. Landmarks are kept as segment SUMS (the
/64 is folded into the exp scale).

Newton-Schulz pseudo-inverse is reformulated on N = (1/c) Km^T Km,
which is symmetric, so the whole iteration needs no transposes:
  N_{k+1} = 0.25 N_k Qp(N_k),  Qp(X) = 13I - 15X + 7X^2 - X^3
  R = prod_k 0.25 Qp(N_k)  =>  Vi6 = (1/c) R Km^T
  W = Vi6 @ (diag(1/r3) G) = (1/c) R @ (Km^T G~)
The reference's init scale c = max over ALL (b,h) of colsums of
kernel_2 couples the shards; we compute c exactly on the host (cheap
numpy reduction producing one scalar) and pass 1/c as a tiny input.
"""

import numpy as np

B, H, S, D, L = 8, 12, 4096, 64, 64
NCORES = 8
PER_CORE = (B * H) // NCORES      # 12 slices
NPAIRS = PER_CORE // 2            # 6
NBLK = S // 512                   # 8 blocks of 512 rows
NCHUNK = S // 128                 # 32 chunks (bb, t)
SCALE2 = 0.125                    # (d^-1/4)^2
EXP_SCALE_SL = SCALE2 / 64.0      # for S1, S3 logits (one landmark-sum side)
EXP_SCALE_S2 = SCALE2 / 4096.0    # for S2 logits (two landmark-sum sides)

# fp16 consts column layout
C_I128 = 0        # [128,128] identity (I|ACOL must be adjacent)
C_ACOL = 128      # [128,8] landmark indicator cols (16-row bands)
C_ONES = 136      # [128,1] ones column (r3 reduction lhsT)
C_NCOLS = 137
# fp32 consts
C32_I13 = 0       # [128,64] 13*[I64;I64]
C32_P15 = 64      # [128,128] 15*I
C32_M7 = 192      # [128,128] -7*I
C32_I65 = 320     # [128,65] I65 in rows 0:65
C32_NCOLS = 385

_PROG_CACHE = {}


def _make_consts():
    C = np.zeros((128, C_NCOLS), np.float16)
    I128 = np.eye(128, dtype=np.float16)
    C[:, C_I128:C_I128 + 128] = I128
    for j in range(8):
        C[16 * j:16 * j + 16, C_ACOL + j] = 1.0
    C[:, C_ONES] = 1.0
    C32 = np.zeros((128, C32_NCOLS), np.float32)
    I64 = np.eye(64, dtype=np.float32)
    C32[0:64, C32_I13:C32_I13 + 64] = 13.0 * I64
    C32[64:128, C32_I13:C32_I13 + 64] = 13.0 * I64
    I128f = np.eye(128, dtype=np.float32)
    C32[:, C32_P15:C32_P15 + 128] = 15.0 * I128f
    C32[:, C32_M7:C32_M7 + 128] = -7.0 * I128f
    C32[0:65, C32_I65:C32_I65 + 65] = np.eye(65, dtype=np.float32)
    return C, C32


def _host_global_c(Q, K, mask):
    """Exact global max of kernel_2 column-sums (one fp32 scalar)."""
    scale = np.float32(1.0 / np.sqrt(np.sqrt(D)))
    if mask.min() >= 1.0 and mask.max() <= 1.0:
        Qs = Q
        Ks = K
    else:
        m = mask[:, None, :, None].astype(np.float32)
        Qs = Q * m
        Ks = K * m
    seg = S // L
    Q_l = Qs.reshape(B, H, L, seg, D).mean(axis=-2, dtype=np.float32) * scale
    K_l = Ks.reshape(B, H, L, seg, D).mean(axis=-2, dtype=np.float32) * scale
    s2 = np.einsum('bhld,bhmd->bhlm', Q_l, K_l).astype(np.float32)
    s2 -= s2.max(axis=-1, keepdims=True)
    e = np.exp(s2, dtype=np.float32)
    k2 = e / e.sum(axis=-1, keepdims=True, dtype=np.float32)
    return np.float32(k2.sum(axis=-2, dtype=np.float32).max())


def _build_program(npairs=NPAIRS, debug=False, ones_mask=True):
    import concourse.bacc as bacc
    import concourse.mybir as mybir
    import concourse.tile as tile
    from concourse.bass import ds

    f32 = mybir.dt.float32
    f16 = mybir.dt.float16
    AF = mybir.ActivationFunctionType
    AX = mybir.AxisListType
    OP = mybir.AluOpType

    per_core = npairs * 2
    nc = bacc.Bacc("TRN2", target_bir_lowering=False, debug=False)
    qd = nc.dram_tensor("q", [npairs, S, 128], f32, kind="ExternalInput").ap()
    kd = nc.dram_tensor("k", [npairs, S, 128], f32, kind="ExternalInput").ap()
    vd = nc.dram_tensor("v", [per_core, S, 68], f32, kind="ExternalInput").ap()
    rcd = nc.dram_tensor("rc", [128, 1], f32, kind="ExternalInput").ap()
    cd = nc.dram_tensor("c", [128, C_NCOLS], f16, kind="ExternalInput").ap()
    cd32 = nc.dram_tensor("c32", [128, C32_NCOLS], f32,
                          kind="ExternalInput").ap()
    xd = nc.dram_tensor("x", [per_core, S, D], f32, kind="ExternalOutput").ap()

    with tile.TileContext(nc) as tc:
        with (
            tc.tile_pool(name="cst", bufs=1) as cpool,
            tc.tile_pool(name="bigT", bufs=2) as bigT,
            tc.tile_pool(name="med", bufs=4) as med,
            tc.tile_pool(name="sml", bufs=2) as sml,
            tc.tile_pool(name="psA", bufs=3, space="PSUM") as psA,
            tc.tile_pool(name="psB", bufs=2, space="PSUM") as psB,
            tc.tile_pool(name="psC", bufs=3, space="PSUM") as psC,
        ):
            cst = cpool.tile([128, C_NCOLS], f16)
            nc.sync.dma_start(out=cst, in_=cd)
            cst32 = cpool.tile([128, C32_NCOLS], f32)
            nc.sync.dma_start(out=cst32, in_=cd32)
            rcb = cpool.tile([128, 1], f32)
            nc.sync.dma_start(out=rcb, in_=rcd)
            IA = cst[:, C_I128:C_I128 + 136]     # [I128 | ACOL8] fp16
            I13 = cst32[:, C32_I13:C32_I13 + 64]
            I65 = cst32[0:65, C32_I65:C32_I65 + 65]

            # ============================================================
            # The PE executes its queue IN ORDER: any matmul that waits on
            # a DVE/ACT round trip blocks every matmul behind it, the PE
            # micro-idles, and HAM re-throttles the array to 1.2 GHz.  So
            # everything serial (Newton-Schulz, S2->km, the W chain, the
            # G/X finalizes) is chopped into small parts and emitted spread
            # out BETWEEN the dense fp16 matmul groups, and the per-pair
            # phases are software-pipelined across pairs:
            #   pair p emission: ingest(p), T(p), [W-chain(p-1)], S2(p),
            #                    [X(p-1)], E3G/E1(p) x NS-parts(p)
            # ============================================================

            def emit_ingest(p, st):
                a, b = 2 * p, 2 * p + 1
                st["ntq"] = bigT.tile([128, 4096], f16, tag="ntq",
                                      name=f"ntq{p}")
                st["ntk"] = bigT.tile([128, 4096], f16, tag="ntk",
                                      name=f"ntk{p}")
                for srcd, nt in ((qd, st["ntq"]), (kd, st["ntk"])):
                    nc.gpsimd.dma_start(
                        out=nt.rearrange("p (bb c) -> p bb c", bb=NBLK),
                        in_=srcd[p].rearrange("(bb p t) c -> p bb (t c)",
                                              bb=NBLK, p=128))
                st["vva"] = bigT.tile([128, 2176], f16, tag="vva",
                                      name=f"vva{p}")
                st["vvb"] = bigT.tile([128, 2176], f16, tag="vvb",
                                      name=f"vvb{p}")
                for sl, vv in ((a, st["vva"]), (b, st["vvb"])):
                    nc.gpsimd.dma_start(
                        out=vv.rearrange("p (bb c) -> p bb c", bb=NBLK),
                        in_=vd[sl].rearrange("(bb p t) c -> p bb (t c)",
                                             bb=NBLK, p=128))

            def emit_T(p, st):
                # fused transpose + landmark partial sums; chunk c = 4bb+t
                st["qts"] = bigT.tile([128, 4096], f16, tag="qts",
                                      name=f"qts{p}")
                st["kts"] = bigT.tile([128, 4096], f16, tag="kts",
                                      name=f"kts{p}")
                st["pq"] = sml.tile([128, 256], f32, tag="pq", name=f"pq{p}")
                st["pk"] = sml.tile([128, 256], f32, tag="pk", name=f"pk{p}")
                for ti, (nt, dst, pstrip) in enumerate(
                        ((st["ntq"], st["qts"], st["pq"]),
                         (st["ntk"], st["kts"], st["pk"]))):
                    for g in range(11):  # 3 chunks per psum bank (last: 2)
                        n_in_g = 3 if g < 10 else 2
                        pst = psA.tile([128, 512], f32, tag="bigps",
                                       name=f"pst{p}_{ti}_{g}")
                        for k in range(n_in_g):
                            c = 3 * g + k
                            nc.tensor.matmul(
                                pst[:, ds(136 * k, 136)],
                                nt[:, ds(128 * c, 128)], IA,
                                start=True, stop=True,
                                skip_group_check=True)
                        pstv = pst[:, 0:136 * n_in_g] \
                            .rearrange("p (k w) -> p k w", w=136)
                        tcp = dst[:, ds(384 * g, 128 * n_in_g)] \
                            .rearrange("p (k w) -> p k w", w=128)
                        pcp = pstrip[:, ds(24 * g, 8 * n_in_g)] \
                            .rearrange("p (k w) -> p k w", w=8)
                        if (ti + g) % 2 == 0:
                            nc.vector.tensor_copy(tcp,
                                                  pstv[:, 0:n_in_g, 0:128])
                            nc.scalar.copy(out=pcp,
                                           in_=pstv[:, 0:n_in_g, 128:136])
                        else:
                            nc.scalar.copy(out=tcp,
                                           in_=pstv[:, 0:n_in_g, 0:128])
                            nc.vector.tensor_copy(pcp,
                                                  pstv[:, 0:n_in_g, 128:136])

            def emit_bd(p, st):
                # landmark t-sums (fp32) + blockdiag tiles
                st["lmq"] = sml.tile([128, 64], f32, tag="lmq",
                                     name=f"lmq{p}")
                st["lmk"] = sml.tile([128, 64], f32, tag="lmk",
                                     name=f"lmk{p}")
                for ti, (pstrip, lm) in enumerate(
                        ((st["pq"], st["lmq"]), (st["pk"], st["lmk"]))):
                    lv = pstrip.rearrange("p (bb t j) -> p bb t j",
                                          bb=NBLK, t=4)
                    t01 = sml.tile([128, 64], f32, tag="t01",
                                   name=f"t01{p}_{ti}")
                    t01v = t01.rearrange("p (bb j) -> p bb j", bb=NBLK)
                    nc.vector.tensor_add(t01v, lv[:, :, 0, :], lv[:, :, 1, :])
                    t23 = sml.tile([128, 64], f32, tag="t23",
                                   name=f"t23{p}_{ti}")
                    t23v = t23.rearrange("p (bb j) -> p bb j", bb=NBLK)
                    nc.vector.tensor_add(t23v, lv[:, :, 2, :], lv[:, :, 3, :])
                    nc.vector.tensor_add(lm, t01, t23)
                bdq = sml.tile([128, 128], f32, tag="bdq", name=f"bdq{p}")
                bdk = sml.tile([128, 128], f32, tag="bdk", name=f"bdk{p}")
                for bd, lm in ((bdq, st["lmq"]), (bdk, st["lmk"])):
                    nc.gpsimd.memset(bd[0:64, 64:128], 0.0)
                    nc.gpsimd.memset(bd[64:128, 0:64], 0.0)
                    nc.vector.tensor_copy(bd[0:64, 0:64], lm[0:64, :])
                    nc.vector.tensor_copy(bd[64:128, 64:128], lm[64:128, :])
                st["bdq"], st["bdk"] = bdq, bdk
                st["bdq16"] = sml.tile([128, 128], f16, tag="bdq16",
                                       name=f"bdq16{p}")
                st["bdk16"] = sml.tile([128, 128], f16, tag="bdk16",
                                       name=f"bdk16{p}")
                nc.vector.tensor_copy(st["bdq16"], bdq)
                nc.scalar.copy(out=st["bdk16"], in_=bdk)

            def emit_s2(p, st):
                ps_s2 = psC.tile([128, 512], f32, tag="xinv",
                                 name=f"pss2{p}")
                nc.tensor.matmul(ps_s2[0:64, 0:64], st["bdq"][0:64, 0:64],
                                 st["bdk"][0:64, 0:64], start=True, stop=True,
                                 tile_position=(0, 0))
                nc.tensor.matmul(ps_s2[64:128, 0:64],
                                 st["bdq"][64:128, 64:128],
                                 st["bdk"][64:128, 64:128],
                                 start=True, stop=True,
                                 tile_position=(64, 64))
                st["ps_s2"] = ps_s2

            # --- Newton-Schulz, chopped into queue-friendly parts -------
            def ns_km(p, st):
                # exp/softmax of S2 -> km, then N0 matmuls + scale
                e2 = sml.tile([128, 64], f32, tag="e2", name=f"e2{p}")
                nc.scalar.activation(e2, st["ps_s2"][:, 0:64], AF.Exp,
                                     scale=EXP_SCALE_S2)
                r2 = sml.tile([128, 1], f32, tag="r2", name=f"r2{p}")
                nc.vector.reduce_sum(r2, e2, axis=AX.X)
                nc.vector.reciprocal(r2, r2)
                km = sml.tile([128, 64], f32, tag="km", name=f"km{p}")
                nc.vector.tensor_mul(km, e2, r2.broadcast_to([128, 64]))
                st["km"] = km
                ps_n0 = psC.tile([128, 512], f32, tag="xinv",
                                 name=f"psn0{p}")
                nc.tensor.matmul(ps_n0[0:64, 0:64], km[0:64, :], km[0:64, :],
                                 start=True, stop=True, tile_position=(0, 0))
                nc.tensor.matmul(ps_n0[64:128, 0:64], km[64:128, :],
                                 km[64:128, :], start=True, stop=True,
                                 tile_position=(64, 64))
                n_st = sml.tile([128, 64], f32, tag="nst", name=f"n0{p}")
                nc.vector.tensor_mul(n_st, ps_n0[:, 0:64],
                                     rcb.broadcast_to([128, 64]))
                st["ns_n"] = n_st

            def ns_sq(p, st, it):
                n_st = st["ns_n"]
                ps_sq = psC.tile([128, 512], f32, tag="xinv",
                                 name=f"psq{p}_{it}")
                nc.tensor.matmul(ps_sq[0:64, 0:64], n_st[0:64, :],
                                 n_st[0:64, :], start=True, stop=True,
                                 tile_position=(0, 0))
                nc.tensor.matmul(ps_sq[64:128, 0:64], n_st[64:128, :],
                                 n_st[64:128, :], start=True, stop=True,
                                 tile_position=(64, 64))
                n2 = sml.tile([128, 64], f32, tag="n2", name=f"n2{p}_{it}")
                nc.vector.tensor_copy(n2, ps_sq[:, 0:64])
                st["ns_n2"] = n2

            def ns_n3(p, st, it):
                n_st, n2 = st["ns_n"], st["ns_n2"]
                ps_qp = psC.tile([128, 512], f32, tag="xinv",
                                 name=f"psqp{p}_{it}")
                nc.tensor.matmul(ps_qp[0:64, 0:64], n_st[0:64, :],
                                 n2[0:64, :], start=True, stop=True,
                                 tile_position=(0, 0),
                                 skip_group_check=True)
                nc.tensor.matmul(ps_qp[64:128, 0:64], n_st[64:128, :],
                                 n2[64:128, :], start=True, stop=True,
                                 tile_position=(64, 64),
                                 skip_group_check=True)
                # u = 15*N + N^3 ; v = -7*N^2 + u ; qp = v - 13I
                u_t = sml.tile([128, 64], f32, tag="ut", name=f"u{p}_{it}")
                nc.vector.scalar_tensor_tensor(
                    u_t, n_st, 15.0, ps_qp[:, 0:64],
                    op0=OP.mult, op1=OP.add)
                v_t = sml.tile([128, 64], f32, tag="vt", name=f"v{p}_{it}")
                nc.vector.scalar_tensor_tensor(
                    v_t, n2, -7.0, u_t, op0=OP.mult, op1=OP.add)
                qp = sml.tile([128, 64], f32, tag="qp", name=f"qp{p}_{it}")
                nc.vector.tensor_sub(qp, v_t, I13)
                st["ns_qp"] = qp

            def ns_rn(p, st, it):
                n_st, qp, r_st = st["ns_n"], st["ns_qp"], st["ns_r"]
                if it == 0:
                    r_new = sml.tile([128, 64], f32, tag="rst",
                                     name=f"r{p}_{it}")
                    nc.vector.tensor_scalar_mul(r_new, qp, -0.25)
                else:
                    ps_r = psC.tile([128, 512], f32, tag="xinv",
                                    name=f"psr{p}_{it}")
                    nc.tensor.matmul(ps_r[0:64, 0:64], r_st[0:64, :],
                                     qp[0:64, :], start=True, stop=True,
                                     tile_position=(0, 0))
                    nc.tensor.matmul(ps_r[64:128, 0:64], r_st[64:128, :],
                                     qp[64:128, :], start=True, stop=True,
                                     tile_position=(64, 64))
                    r_new = sml.tile([128, 64], f32, tag="rst",
                                     name=f"r{p}_{it}")
                    nc.vector.tensor_scalar_mul(r_new, ps_r[:, 0:64], -0.25)
                st["ns_r"] = r_new
                if it < 5:
                    ps_nn = psC.tile([128, 512], f32, tag="xinv",
                                     name=f"psnn{p}_{it}")
                    nc.tensor.matmul(ps_nn[0:64, 0:64], n_st[0:64, :],
                                     qp[0:64, :], start=True, stop=True,
                                     tile_position=(0, 0))
                    nc.tensor.matmul(ps_nn[64:128, 0:64], n_st[64:128, :],
                                     qp[64:128, :], start=True, stop=True,
                                     tile_position=(64, 64))
                    n_new = sml.tile([128, 64], f32, tag="nst",
                                     name=f"n{p}_{it}")
                    nc.vector.tensor_scalar_mul(n_new, ps_nn[:, 0:64], -0.25)
                    st["ns_n"] = n_new

            def emit_wchain(p, st):
                # G finalize: transpose G^T (+r3 row) back to [l, (d|r3)]
                gts = sml.tile([128, 128], f32, tag="gts", name=f"gts{p}")
                nc.vector.tensor_copy(gts[0:65, 0:64],
                                      st["ps_ga"][0:65, 0:64])
                nc.vector.tensor_copy(gts[0:65, 64:128],
                                      st["ps_gb"][0:65, 0:64])
                ps_g2 = psC.tile([128, 512], f32, tag="xinv",
                                 name=f"psg2{p}")
                nc.tensor.matmul(ps_g2[:, 0:65], gts[0:65, 0:128], I65,
                                 start=True, stop=True)
                r3r = sml.tile([128, 1], f32, tag="r3", name=f"r3{p}")
                nc.vector.reciprocal(r3r, ps_g2[:, 64:65])
                gt = sml.tile([128, 64], f32, tag="gt", name=f"gt{p}")
                nc.vector.tensor_mul(gt, ps_g2[:, 0:64],
                                     r3r.broadcast_to([128, 64]))
                st["gt"] = gt

            def emit_kg(p, st):
                km, gt = st["km"], st["gt"]
                ps_kg = psC.tile([128, 512], f32, tag="xinv",
                                 name=f"pskg{p}")
                nc.tensor.matmul(ps_kg[0:64, 0:64], km[0:64, :], gt[0:64, :],
                                 start=True, stop=True, tile_position=(0, 0))
                nc.tensor.matmul(ps_kg[64:128, 0:64], km[64:128, :],
                                 gt[64:128, :], start=True, stop=True,
                                 tile_position=(64, 64))
                kg = sml.tile([128, 64], f32, tag="kg", name=f"kg{p}")
                nc.vector.tensor_copy(kg, ps_kg[:, 0:64])
                st["kg"] = kg

            def emit_w(p, st):
                kg, r_st = st["kg"], st["ns_r"]
                ps_w = psC.tile([128, 512], f32, tag="xinv", name=f"psw{p}")
                nc.tensor.matmul(ps_w[0:64, 0:64], r_st[0:64, :], kg[0:64, :],
                                 start=True, stop=True, tile_position=(0, 0))
                nc.tensor.matmul(ps_w[64:128, 0:64], r_st[64:128, :],
                                 kg[64:128, :], start=True, stop=True,
                                 tile_position=(64, 64))
                wbd = sml.tile([128, 130], f16, tag="wbd", name=f"wbd{p}")
                nc.gpsimd.memset(wbd[0:64, 65:130], 0.0)
                nc.gpsimd.memset(wbd[64:128, 0:65], 0.0)
                nc.gpsimd.memset(wbd[0:64, 64:65], 1.0)
                nc.gpsimd.memset(wbd[64:128, 129:130], 1.0)
                nc.vector.tensor_mul(wbd[0:64, 0:64], ps_w[0:64, 0:64],
                                     rcb[0:64, :].broadcast_to([64, 64]))
                nc.vector.tensor_mul(wbd[64:128, 65:129], ps_w[64:128, 0:64],
                                     rcb[64:128, :].broadcast_to([64, 64]))
                st["wbd"] = wbd

            def x_parts_of(p, st):
                # X phase as a list of small closures so it can thread
                # through the next pair's dense fp16 loop (one psum-group
                # of 2 chunks, or one store DMA, per part).
                a, b = 2 * p, 2 * p + 1
                e1t, wbd = st["e1t"], st["wbd"]
                parts = []

                def mk_group(u, k):
                    def f():
                        if k == 0:
                            st[f"xo{u}"] = med.tile([128, 1024], f32,
                                                    tag="xo",
                                                    name=f"xo{p}_{u}")
                        xo = st[f"xo{u}"]
                        xov = xo.rearrange("p (h bb t d) -> p h bb t d",
                                           h=2, bb=2, t=4)
                        ps_x = psC.tile([128, 512], f32, tag="xinv",
                                        name=f"psx{p}_{u}_{k}")
                        for r in range(2):
                            c = 8 * u + 2 * k + r
                            nc.tensor.matmul(
                                ps_x[:, ds(130 * r, 130)],
                                e1t[:, ds(128 * c, 128)], wbd,
                                start=True, stop=True,
                                skip_group_check=True)
                        psxv = ps_x[:, 0:260].rearrange(
                            "p (r h w) -> p r h w", r=2, h=2)
                        rr = sml.tile([128, 4], f32, tag="rr",
                                      name=f"rr{p}_{u}_{k}")
                        rrv = rr.rearrange("p (r h) -> p r h", r=2)
                        nc.vector.reciprocal(
                            rrv, psxv[:, :, :, 64:65]
                            .rearrange("p r h one -> p r (h one)"))
                        bb, t0 = (2 * k) // 4, (2 * k) % 4
                        nc.vector.tensor_mul(
                            xov[:, :, bb, t0:t0 + 2, :],
                            psxv[:, :, :, 0:64]
                            .rearrange("p r h d -> p h r d"),
                            rrv.rearrange("p r h -> p h r")[:, :, :, None]
                            .broadcast_to([128, 2, 2, 64]))
                    return f

                def mk_store(u, h, sl):
                    def f():
                        xo = st[f"xo{u}"]
                        nc.sync.dma_start(
                            out=xd[sl, ds(1024 * u, 1024), :]
                            .rearrange("(bb p t) d -> p bb (t d)",
                                       bb=2, p=128),
                            in_=xo.rearrange("p (h c) -> p h c", h=2)[:, h]
                            .rearrange("p (bb c) -> p bb c", bb=2))
                    return f

                for u in range(4):
                    for k in range(4):
                        parts.append(mk_group(u, k))
                    parts.append(mk_store(u, 0, a))
                    parts.append(mk_store(u, 1, b))
                return parts

            def emit_x(p, st):
                for f in x_parts_of(p, st):
                    f()

            def emit_e3g_loop(p, st, xparts):
                # dense fp16 stream: E3 groups + E1 groups + (one group
                # late) G matmuls, with the serial NS parts dropped in
                # between so their DVE round trips hide under fp16 MMs.
                st["ps_ga"] = psB.tile([128, 512], f32, tag="gacc",
                                       name=f"psga{p}")
                st["ps_gb"] = psB.tile([128, 512], f32, tag="gacc",
                                       name=f"psgb{p}")
                st["e1t"] = bigT.tile([128, 4096], f16, tag="e1t",
                                      name=f"e1t{p}")
                kts, qts = st["kts"], st["qts"]
                vva, vvb = st["vva"], st["vvb"]
                bdq16, bdk16 = st["bdq16"], st["bdk16"]
                e1t = st["e1t"]

                parts = [lambda: ns_km(p, st)]
                for it in range(6):
                    parts.append(lambda it=it: ns_sq(p, st, it))
                    parts.append(lambda it=it: ns_n3(p, st, it))
                    parts.append(lambda it=it: ns_rn(p, st, it))

                def pop_part():
                    if parts:
                        parts.pop(0)()

                def pop_x():
                    if xparts:
                        xparts.pop(0)()

                def emit_g(g, e3t):
                    gw = 64 if ones_mask else 65
                    for ci in range(4):
                        c = 4 * g + ci
                        first, last = (c == 0), (c == NCHUNK - 1)
                        vcol = 272 * g + 68 * ci  # (bb=g, t=ci) chunk
                        nc.tensor.matmul(
                            st["ps_ga"][0:65, 0:64],
                            vva[:, ds(vcol, 65)],
                            e3t[:, ds(128 * ci, 64)],
                            start=first, stop=last,
                            tile_position=(0, 0), skip_group_check=True)
                        nc.tensor.matmul(
                            st["ps_gb"][0:65, 0:64],
                            vvb[:, ds(vcol, 65)],
                            e3t[:, ds(128 * ci + 64, 64)],
                            start=first, stop=last,
                            tile_position=(0, 0), skip_group_check=True)

                prev_e3t = None
                for g in range(8):
                    ps_e3 = psA.tile([128, 512], f32, tag="bigps",
                                     name=f"pse3{p}_{g}")
                    for ci in range(4):
                        c = 4 * g + ci
                        nc.tensor.matmul(ps_e3[:, ds(128 * ci, 128)],
                                         kts[:, ds(128 * c, 128)], bdq16,
                                         start=True, stop=True,
                                         skip_group_check=True)
                    e3t = med.tile([128, 512], f16, tag="e3t",
                                   name=f"e3t{p}_{g}")
                    nc.scalar.activation(e3t, ps_e3, AF.Exp,
                                         scale=EXP_SCALE_SL)
                    pop_part()
                    pop_x()
                    ps_s1 = psA.tile([128, 512], f32, tag="bigps",
                                     name=f"pss1{p}_{g}")
                    nc.tensor.matmul(ps_s1, bdk16, qts[:, ds(512 * g, 512)],
                                     start=True, stop=True)
                    nc.scalar.activation(e1t[:, ds(512 * g, 512)], ps_s1,
                                         AF.Exp, scale=EXP_SCALE_SL)
                    if prev_e3t is not None:
                        emit_g(g - 1, prev_e3t)
                    prev_e3t = e3t
                    pop_part()
                    pop_x()
                    if g in (3, 5, 7):
                        pop_part()
                    pop_x()
                emit_g(7, prev_e3t)
                while parts:
                    pop_part()
                while xparts:
                    xparts.pop(0)()

            # ---------------- pipelined pair loop -----------------------
            prev = None
            for p in range(npairs):
                st = {"ns_r": None}
                emit_ingest(p, st)
                emit_T(p, st)
                emit_bd(p, st)
                if prev is not None:
                    emit_wchain(prev["p"], prev)
                    emit_kg(prev["p"], prev)
                emit_s2(p, st)
                xparts = []
                if prev is not None:
                    emit_w(prev["p"], prev)
                    xparts = x_parts_of(prev["p"], prev)
                emit_e3g_loop(p, st, xparts)
                st["p"] = p
                prev = st
            emit_wchain(prev["p"], prev)
            emit_kg(prev["p"], prev)
            emit_w(prev["p"], prev)
            emit_x(prev["p"], prev)
    return nc


def _get_program(npairs=NPAIRS, debug=False, ones_mask=True):
    key = (npairs, debug, ones_mask)
    if key not in _PROG_CACHE:
        nc = _build_program(npairs, debug, ones_mask)
        if not nc.is_finalized():
            nc.finalize()  # Bacc defers register allocation until finalize
        _PROG_CACHE[key] = nc
    return _PROG_CACHE[key]


def run(inputs, trace=False, trace_kwargs=None, debug=False):
    from concourse import bass_utils
    Q, K, V, mask = (np.asarray(inputs["Q"], np.float32),
                     np.asarray(inputs["K"], np.float32),
                     np.asarray(inputs["V"], np.float32),
                     np.asarray(inputs["mask"], np.float32))
    ones_mask = bool(mask.min() >= 1.0 and mask.max() <= 1.0)
    rc = np.full((128, 1), 1.0 / _host_global_c(Q, K, mask), np.float32)
    consts16, consts32 = _make_consts()

    if ones_mask:
        Qm, Km, Vm = Q, K, V
    else:
        m = mask[:, None, :, None].astype(np.float32)
        Qm, Km, Vm = Q * m, K * m, V * m

    npair_tot = (B * H) // 2
    # pair-interleaved [48, S, 128]: row s = [T_a[s] | T_b[s]]
    Qp = np.ascontiguousarray(
        Qm.reshape(npair_tot, 2, S, D).transpose(0, 2, 1, 3)
        .reshape(npair_tot, S, 128))
    Kp = np.ascontiguousarray(
        Km.reshape(npair_tot, 2, S, D).transpose(0, 2, 1, 3)
        .reshape(npair_tot, S, 128))
    # V with mask appended as column 64, padded to 68 so each DRAM row
    # is 272B (16B-aligned descriptor starts): [96, S, 68]
    Vx = np.zeros((B * H, S, 68), np.float32)
    Vx[:, :, :64] = Vm.reshape(B * H, S, D)
    Vx[:, :, 64] = np.broadcast_to(mask[:, None, :], (B, H, S)) \
        .reshape(B * H, S)

    nc = _get_program(debug=debug, ones_mask=ones_mask)
    in_maps = []
    for c in range(NCORES):
        in_maps.append({
            "q": Qp[c * NPAIRS:(c + 1) * NPAIRS],
            "k": Kp[c * NPAIRS:(c + 1) * NPAIRS],
            "v": Vx[c * PER_CORE:(c + 1) * PER_CORE],
            "rc": rc,
            "c": consts16,
            "c32": consts32,
        })
    res = bass_utils.run_bass_kernel_spmd(
        nc, in_maps, core_ids=list(range(NCORES)), trace=trace,
        **(trace_kwargs or {}))
    X = np.concatenate([r["x"] for r in res.results], axis=0)
    return X.reshape(B, H, S, D), res


def kernel(**inputs):
    X, _ = run(inputs, trace=False)
    return X


if __name__ == "__main__":
    # quick build check
    prog = _get_program()
    print("built ok")


# revision 32
# speedup vs baseline: 1.2730x; 1.0031x over previous
"""NystromAttention Trainium2 Bass kernel (SPMD over 8 NeuronCores).

Sharding: (B,H)=96 slices flattened; core i takes slices [12i, 12i+12),
processed as 6 pairs stacked on the 128-partition dim.

v3 design (vs the 517us fp32 baseline, which was PE-bound at 93.5% with
fp32 4-cycle/row matmuls and 256B DMA descriptors):

- fp16 datapath for every BIG matmul (1 cycle/row on the PE instead of
  fp32's 4, plus fast-weight-load). fp32->fp16 cast happens inside the
  SWDGE ingest DMAs (gpsimd dma_start casts for free).
- The landmark->kernel_2->Newton-Schulz->W chain stays fp32: errors in
  the matrix being pseudo-inverted (and in the R/W product chain) are
  amplified by its conditioning; fp16 there costs 5e-2 rel error
  (measured in numpy emulation), fp32 chain + fp16 big path = 1.3e-4.
  These are all tiny 64x64 matmuls, so the fp32 4-cycle cost is small.
- Host-side DRAM staging: Q/K stored pair-interleaved [48, S, 128]
  (= [Q_a[s] | Q_b[s]] per row) and V stored [96, S, 65] with the mask
  appended as column 64. Ingest DMA runs become 2KB contiguous on the
  DRAM side (375 GB/s class vs 213 GB/s at 256B runs), and the
  [a|b]-fused transpose chunks / [V|mask] G-matmul lhsT become single
  contiguous windows (walrus wants 1-free-dim matmul operands).
- Quad-interleaved s-permutation: within each 512-row block, SBUF
  column 128*t + p holds DRAM row 4*p + t. Carried through all
  intermediate tensors and undone in the output store (1KB store runs).
- Landmark segment sums fused into the transpose matmuls:
  rhs = [I128 | ACOL8] (N=136), partials split off to an fp32 strip
  during the PSUM->SBUF copy and summed on DVE. Kills the separate
  per-chunk landmark matmul + its duplicate weight load.
- r3 (kernel_3 row sums) fused into the G matmuls via the 65-column
  [V | mask] lhsT. Kills the per-chunk mask-row matmuls.

All softmaxes skip max-subtraction (logits are ~N(0, 0.125)). Scales
are folded into the ACT exp. Landmarks are kept as segment SUMS (the
/64 is folded into the exp scale).

Newton-Schulz pseudo-inverse is reformulated on N = (1/c) Km^T Km,
which is symmetric, so the whole iteration needs no transposes:
  N_{k+1} = 0.25 N_k Qp(N_k),  Qp(X) = 13I - 15X + 7X^2 - X^3
  R = prod_k 0.25 Qp(N_k)  =>  Vi6 = (1/c) R Km^T
  W = Vi6 @ (diag(1/r3) G) = (1/c) R @ (Km^T G~)
The reference's init scale c = max over ALL (b,h) of colsums of
kernel_2 couples the shards; we compute c exactly on the host (cheap
numpy reduction producing one scalar) and pass 1/c as a tiny input.
"""

import numpy as np

B, H, S, D, L = 8, 12, 4096, 64, 64
NCORES = 8
PER_CORE = (B * H) // NCORES      # 12 slices
NPAIRS = PER_CORE // 2            # 6
NBLK = S // 512                   # 8 blocks of 512 rows
NCHUNK = S // 128                 # 32 chunks (bb, t)
SCALE2 = 0.125                    # (d^-1/4)^2
EXP_SCALE_SL = SCALE2 / 64.0      # for S1, S3 logits (one landmark-sum side)
EXP_SCALE_S2 = SCALE2 / 4096.0    # for S2 logits (two landmark-sum sides)

# fp16 consts column layout
C_I128 = 0        # [128,128] identity (I|ACOL must be adjacent)
C_ACOL = 128      # [128,8] landmark indicator cols (16-row bands)
C_ONES = 136      # [128,1] ones column (r3 reduction lhsT)
C_NCOLS = 137
# fp32 consts
C32_I13 = 0       # [128,64] 13*[I64;I64]
C32_P15 = 64      # [128,128] 15*I
C32_M7 = 192      # [128,128] -7*I
C32_I65 = 320     # [128,65] I65 in rows 0:65
C32_NCOLS = 385

_PROG_CACHE = {}


def _make_consts():
    C = np.zeros((128, C_NCOLS), np.float16)
    I128 = np.eye(128, dtype=np.float16)
    C[:, C_I128:C_I128 + 128] = I128
    for j in range(8):
        C[16 * j:16 * j + 16, C_ACOL + j] = 1.0
    C[:, C_ONES] = 1.0
    C32 = np.zeros((128, C32_NCOLS), np.float32)
    I64 = np.eye(64, dtype=np.float32)
    C32[0:64, C32_I13:C32_I13 + 64] = 13.0 * I64
    C32[64:128, C32_I13:C32_I13 + 64] = 13.0 * I64
    I128f = np.eye(128, dtype=np.float32)
    C32[:, C32_P15:C32_P15 + 128] = 15.0 * I128f
    C32[:, C32_M7:C32_M7 + 128] = -7.0 * I128f
    C32[0:65, C32_I65:C32_I65 + 65] = np.eye(65, dtype=np.float32)
    return C, C32


def _host_global_c(Q, K, mask):
    """Exact global max of kernel_2 column-sums (one fp32 scalar)."""
    scale = np.float32(1.0 / np.sqrt(np.sqrt(D)))
    if mask.min() >= 1.0 and mask.max() <= 1.0:
        Qs = Q
        Ks = K
    else:
        m = mask[:, None, :, None].astype(np.float32)
        Qs = Q * m
        Ks = K * m
    seg = S // L
    Q_l = Qs.reshape(B, H, L, seg, D).mean(axis=-2, dtype=np.float32) * scale
    K_l = Ks.reshape(B, H, L, seg, D).mean(axis=-2, dtype=np.float32) * scale
    s2 = np.einsum('bhld,bhmd->bhlm', Q_l, K_l).astype(np.float32)
    s2 -= s2.max(axis=-1, keepdims=True)
    e = np.exp(s2, dtype=np.float32)
    k2 = e / e.sum(axis=-1, keepdims=True, dtype=np.float32)
    return np.float32(k2.sum(axis=-2, dtype=np.float32).max())


def _build_program(npairs=NPAIRS, debug=False, ones_mask=True):
    import concourse.bacc as bacc
    import concourse.mybir as mybir
    import concourse.tile as tile
    from concourse.bass import ds

    f32 = mybir.dt.float32
    f16 = mybir.dt.float16
    AF = mybir.ActivationFunctionType
    AX = mybir.AxisListType
    OP = mybir.AluOpType

    per_core = npairs * 2
    nc = bacc.Bacc("TRN2", target_bir_lowering=False, debug=False)
    qd = nc.dram_tensor("q", [npairs, S, 128], f32, kind="ExternalInput").ap()
    kd = nc.dram_tensor("k", [npairs, S, 128], f32, kind="ExternalInput").ap()
    vd = nc.dram_tensor("v", [per_core, S, 68], f32, kind="ExternalInput").ap()
    rcd = nc.dram_tensor("rc", [128, 1], f32, kind="ExternalInput").ap()
    cd = nc.dram_tensor("c", [128, C_NCOLS], f16, kind="ExternalInput").ap()
    cd32 = nc.dram_tensor("c32", [128, C32_NCOLS], f32,
                          kind="ExternalInput").ap()
    xd = nc.dram_tensor("x", [per_core, S, D], f32, kind="ExternalOutput").ap()

    with tile.TileContext(nc) as tc:
        with (
            tc.tile_pool(name="cst", bufs=1) as cpool,
            tc.tile_pool(name="bigT", bufs=3) as bigT,
            tc.tile_pool(name="med", bufs=4) as med,
            tc.tile_pool(name="sml", bufs=2) as sml,
            tc.tile_pool(name="psA", bufs=3, space="PSUM") as psA,
            tc.tile_pool(name="psB", bufs=2, space="PSUM") as psB,
            tc.tile_pool(name="psC", bufs=3, space="PSUM") as psC,
        ):
            cst = cpool.tile([128, C_NCOLS], f16)
            nc.sync.dma_start(out=cst, in_=cd)
            cst32 = cpool.tile([128, C32_NCOLS], f32)
            nc.sync.dma_start(out=cst32, in_=cd32)
            rcb = cpool.tile([128, 1], f32)
            nc.sync.dma_start(out=rcb, in_=rcd)
            IA = cst[:, C_I128:C_I128 + 136]     # [I128 | ACOL8] fp16
            I13 = cst32[:, C32_I13:C32_I13 + 64]
            I65 = cst32[0:65, C32_I65:C32_I65 + 65]

            # ============================================================
            # The PE executes its queue IN ORDER: any matmul that waits on
            # a DVE/ACT round trip blocks every matmul behind it, the PE
            # micro-idles, and HAM re-throttles the array to 1.2 GHz.  So
            # everything serial (Newton-Schulz, S2->km, the W chain, the
            # G/X finalizes) is chopped into small parts and emitted spread
            # out BETWEEN the dense fp16 matmul groups, and the per-pair
            # phases are software-pipelined across pairs:
            #   pair p emission: ingest(p), T(p), [W-chain(p-1)], S2(p),
            #                    [X(p-1)], E3G/E1(p) x NS-parts(p)
            # ============================================================

            def emit_ingest(p, st):
                a, b = 2 * p, 2 * p + 1
                st["ntq"] = bigT.tile([128, 4096], f16, tag="ntq",
                                      name=f"ntq{p}")
                st["ntk"] = bigT.tile([128, 4096], f16, tag="ntk",
                                      name=f"ntk{p}")
                for srcd, nt in ((qd, st["ntq"]), (kd, st["ntk"])):
                    nc.gpsimd.dma_start(
                        out=nt.rearrange("p (bb c) -> p bb c", bb=NBLK),
                        in_=srcd[p].rearrange("(bb p t) c -> p bb (t c)",
                                              bb=NBLK, p=128))
                st["vva"] = bigT.tile([128, 2176], f16, tag="vva",
                                      name=f"vva{p}")
                st["vvb"] = bigT.tile([128, 2176], f16, tag="vvb",
                                      name=f"vvb{p}")
                for sl, vv in ((a, st["vva"]), (b, st["vvb"])):
                    nc.gpsimd.dma_start(
                        out=vv.rearrange("p (bb c) -> p bb c", bb=NBLK),
                        in_=vd[sl].rearrange("(bb p t) c -> p bb (t c)",
                                             bb=NBLK, p=128))

            def emit_T(p, st):
                # fused transpose + landmark partial sums; chunk c = 4bb+t
                st["qts"] = bigT.tile([128, 4096], f16, tag="qts",
                                      name=f"qts{p}")
                st["kts"] = bigT.tile([128, 4096], f16, tag="kts",
                                      name=f"kts{p}")
                st["pq"] = sml.tile([128, 256], f32, tag="pq", name=f"pq{p}")
                st["pk"] = sml.tile([128, 256], f32, tag="pk", name=f"pk{p}")
                for ti, (nt, dst, pstrip) in enumerate(
                        ((st["ntq"], st["qts"], st["pq"]),
                         (st["ntk"], st["kts"], st["pk"]))):
                    for g in range(11):  # 3 chunks per psum bank (last: 2)
                        n_in_g = 3 if g < 10 else 2
                        pst = psA.tile([128, 512], f32, tag="bigps",
                                       name=f"pst{p}_{ti}_{g}")
                        for k in range(n_in_g):
                            c = 3 * g + k
                            nc.tensor.matmul(
                                pst[:, ds(136 * k, 136)],
                                nt[:, ds(128 * c, 128)], IA,
                                start=True, stop=True,
                                skip_group_check=True)
                        pstv = pst[:, 0:136 * n_in_g] \
                            .rearrange("p (k w) -> p k w", w=136)
                        tcp = dst[:, ds(384 * g, 128 * n_in_g)] \
                            .rearrange("p (k w) -> p k w", w=128)
                        pcp = pstrip[:, ds(24 * g, 8 * n_in_g)] \
                            .rearrange("p (k w) -> p k w", w=8)
                        if (ti + g) % 2 == 0:
                            nc.vector.tensor_copy(tcp,
                                                  pstv[:, 0:n_in_g, 0:128])
                            nc.scalar.copy(out=pcp,
                                           in_=pstv[:, 0:n_in_g, 128:136])
                        else:
                            nc.scalar.copy(out=tcp,
                                           in_=pstv[:, 0:n_in_g, 0:128])
                            nc.vector.tensor_copy(pcp,
                                                  pstv[:, 0:n_in_g, 128:136])

            def emit_bd(p, st):
                # landmark t-sums (fp32) + blockdiag tiles
                st["lmq"] = sml.tile([128, 64], f32, tag="lmq",
                                     name=f"lmq{p}")
                st["lmk"] = sml.tile([128, 64], f32, tag="lmk",
                                     name=f"lmk{p}")
                for ti, (pstrip, lm) in enumerate(
                        ((st["pq"], st["lmq"]), (st["pk"], st["lmk"]))):
                    lv = pstrip.rearrange("p (bb t j) -> p bb t j",
                                          bb=NBLK, t=4)
                    t01 = sml.tile([128, 64], f32, tag="t01",
                                   name=f"t01{p}_{ti}")
                    t01v = t01.rearrange("p (bb j) -> p bb j", bb=NBLK)
                    nc.vector.tensor_add(t01v, lv[:, :, 0, :], lv[:, :, 1, :])
                    t23 = sml.tile([128, 64], f32, tag="t23",
                                   name=f"t23{p}_{ti}")
                    t23v = t23.rearrange("p (bb j) -> p bb j", bb=NBLK)
                    nc.vector.tensor_add(t23v, lv[:, :, 2, :], lv[:, :, 3, :])
                    nc.vector.tensor_add(lm, t01, t23)
                bdq = sml.tile([128, 128], f32, tag="bdq", name=f"bdq{p}")
                bdk = sml.tile([128, 128], f32, tag="bdk", name=f"bdk{p}")
                for bd, lm in ((bdq, st["lmq"]), (bdk, st["lmk"])):
                    nc.gpsimd.memset(bd[0:64, 64:128], 0.0)
                    nc.gpsimd.memset(bd[64:128, 0:64], 0.0)
                    nc.vector.tensor_copy(bd[0:64, 0:64], lm[0:64, :])
                    nc.vector.tensor_copy(bd[64:128, 64:128], lm[64:128, :])
                st["bdq"], st["bdk"] = bdq, bdk
                st["bdq16"] = sml.tile([128, 128], f16, tag="bdq16",
                                       name=f"bdq16{p}")
                st["bdk16"] = sml.tile([128, 128], f16, tag="bdk16",
                                       name=f"bdk16{p}")
                nc.vector.tensor_copy(st["bdq16"], bdq)
                nc.scalar.copy(out=st["bdk16"], in_=bdk)

            def emit_s2(p, st):
                ps_s2 = psC.tile([128, 512], f32, tag="xinv",
                                 name=f"pss2{p}")
                nc.tensor.matmul(ps_s2[0:64, 0:64], st["bdq"][0:64, 0:64],
                                 st["bdk"][0:64, 0:64], start=True, stop=True,
                                 tile_position=(0, 0))
                nc.tensor.matmul(ps_s2[64:128, 0:64],
                                 st["bdq"][64:128, 64:128],
                                 st["bdk"][64:128, 64:128],
                                 start=True, stop=True,
                                 tile_position=(64, 64))
                st["ps_s2"] = ps_s2

            # --- Newton-Schulz, chopped into queue-friendly parts -------
            def ns_km(p, st):
                # exp/softmax of S2 -> km, then N0 matmuls + scale
                e2 = sml.tile([128, 64], f32, tag="e2", name=f"e2{p}")
                nc.scalar.activation(e2, st["ps_s2"][:, 0:64], AF.Exp,
                                     scale=EXP_SCALE_S2)
                r2 = sml.tile([128, 1], f32, tag="r2", name=f"r2{p}")
                nc.vector.reduce_sum(r2, e2, axis=AX.X)
                nc.vector.reciprocal(r2, r2)
                km = sml.tile([128, 64], f32, tag="km", name=f"km{p}")
                nc.vector.tensor_mul(km, e2, r2.broadcast_to([128, 64]))
                st["km"] = km
                ps_n0 = psC.tile([128, 512], f32, tag="xinv",
                                 name=f"psn0{p}")
                nc.tensor.matmul(ps_n0[0:64, 0:64], km[0:64, :], km[0:64, :],
                                 start=True, stop=True, tile_position=(0, 0))
                nc.tensor.matmul(ps_n0[64:128, 0:64], km[64:128, :],
                                 km[64:128, :], start=True, stop=True,
                                 tile_position=(64, 64))
                n_st = sml.tile([128, 64], f32, tag="nst", name=f"n0{p}")
                nc.vector.tensor_mul(n_st, ps_n0[:, 0:64],
                                     rcb.broadcast_to([128, 64]))
                st["ns_n"] = n_st

            def ns_sq(p, st, it):
                n_st = st["ns_n"]
                ps_sq = psC.tile([128, 512], f32, tag="xinv",
                                 name=f"psq{p}_{it}")
                nc.tensor.matmul(ps_sq[0:64, 0:64], n_st[0:64, :],
                                 n_st[0:64, :], start=True, stop=True,
                                 tile_position=(0, 0))
                nc.tensor.matmul(ps_sq[64:128, 0:64], n_st[64:128, :],
                                 n_st[64:128, :], start=True, stop=True,
                                 tile_position=(64, 64))
                n2 = sml.tile([128, 64], f32, tag="n2", name=f"n2{p}_{it}")
                nc.vector.tensor_copy(n2, ps_sq[:, 0:64])
                st["ns_n2"] = n2

            def ns_n3(p, st, it):
                n_st, n2 = st["ns_n"], st["ns_n2"]
                ps_qp = psC.tile([128, 512], f32, tag="xinv",
                                 name=f"psqp{p}_{it}")
                nc.tensor.matmul(ps_qp[0:64, 0:64], n_st[0:64, :],
                                 n2[0:64, :], start=True, stop=True,
                                 tile_position=(0, 0),
                                 skip_group_check=True)
                nc.tensor.matmul(ps_qp[64:128, 0:64], n_st[64:128, :],
                                 n2[64:128, :], start=True, stop=True,
                                 tile_position=(64, 64),
                                 skip_group_check=True)
                # u = 15*N + N^3 ; v = -7*N^2 + u ; qp = v - 13I
                u_t = sml.tile([128, 64], f32, tag="ut", name=f"u{p}_{it}")
                nc.vector.scalar_tensor_tensor(
                    u_t, n_st, 15.0, ps_qp[:, 0:64],
                    op0=OP.mult, op1=OP.add)
                v_t = sml.tile([128, 64], f32, tag="vt", name=f"v{p}_{it}")
                nc.vector.scalar_tensor_tensor(
                    v_t, n2, -7.0, u_t, op0=OP.mult, op1=OP.add)
                qp = sml.tile([128, 64], f32, tag="qp", name=f"qp{p}_{it}")
                nc.vector.tensor_sub(qp, v_t, I13)
                st["ns_qp"] = qp

            def ns_rn(p, st, it):
                n_st, qp, r_st = st["ns_n"], st["ns_qp"], st["ns_r"]
                if it == 0:
                    r_new = sml.tile([128, 64], f32, tag="rst",
                                     name=f"r{p}_{it}")
                    nc.vector.tensor_scalar_mul(r_new, qp, -0.25)
                else:
                    ps_r = psC.tile([128, 512], f32, tag="xinv",
                                    name=f"psr{p}_{it}")
                    nc.tensor.matmul(ps_r[0:64, 0:64], r_st[0:64, :],
                                     qp[0:64, :], start=True, stop=True,
                                     tile_position=(0, 0))
                    nc.tensor.matmul(ps_r[64:128, 0:64], r_st[64:128, :],
                                     qp[64:128, :], start=True, stop=True,
                                     tile_position=(64, 64))
                    r_new = sml.tile([128, 64], f32, tag="rst",
                                     name=f"r{p}_{it}")
                    nc.vector.tensor_scalar_mul(r_new, ps_r[:, 0:64], -0.25)
                st["ns_r"] = r_new
                if it < 5:
                    ps_nn = psC.tile([128, 512], f32, tag="xinv",
                                     name=f"psnn{p}_{it}")
                    nc.tensor.matmul(ps_nn[0:64, 0:64], n_st[0:64, :],
                                     qp[0:64, :], start=True, stop=True,
                                     tile_position=(0, 0))
                    nc.tensor.matmul(ps_nn[64:128, 0:64], n_st[64:128, :],
                                     qp[64:128, :], start=True, stop=True,
                                     tile_position=(64, 64))
                    n_new = sml.tile([128, 64], f32, tag="nst",
                                     name=f"n{p}_{it}")
                    nc.vector.tensor_scalar_mul(n_new, ps_nn[:, 0:64], -0.25)
                    st["ns_n"] = n_new

            def emit_wchain(p, st):
                # G finalize: transpose G^T (+r3 row) back to [l, (d|r3)]
                gts = sml.tile([128, 128], f32, tag="gts", name=f"gts{p}")
                nc.vector.tensor_copy(gts[0:65, 0:64],
                                      st["ps_ga"][0:65, 0:64])
                nc.vector.tensor_copy(gts[0:65, 64:128],
                                      st["ps_gb"][0:65, 0:64])
                ps_g2 = psC.tile([128, 512], f32, tag="xinv",
                                 name=f"psg2{p}")
                nc.tensor.matmul(ps_g2[:, 0:65], gts[0:65, 0:128], I65,
                                 start=True, stop=True)
                r3r = sml.tile([128, 1], f32, tag="r3", name=f"r3{p}")
                nc.vector.reciprocal(r3r, ps_g2[:, 64:65])
                gt = sml.tile([128, 64], f32, tag="gt", name=f"gt{p}")
                nc.vector.tensor_mul(gt, ps_g2[:, 0:64],
                                     r3r.broadcast_to([128, 64]))
                st["gt"] = gt

            def emit_kg(p, st):
                km, gt = st["km"], st["gt"]
                ps_kg = psC.tile([128, 512], f32, tag="xinv",
                                 name=f"pskg{p}")
                nc.tensor.matmul(ps_kg[0:64, 0:64], km[0:64, :], gt[0:64, :],
                                 start=True, stop=True, tile_position=(0, 0))
                nc.tensor.matmul(ps_kg[64:128, 0:64], km[64:128, :],
                                 gt[64:128, :], start=True, stop=True,
                                 tile_position=(64, 64))
                kg = sml.tile([128, 64], f32, tag="kg", name=f"kg{p}")
                nc.vector.tensor_copy(kg, ps_kg[:, 0:64])
                st["kg"] = kg

            def emit_w(p, st):
                kg, r_st = st["kg"], st["ns_r"]
                ps_w = psC.tile([128, 512], f32, tag="xinv", name=f"psw{p}")
                nc.tensor.matmul(ps_w[0:64, 0:64], r_st[0:64, :], kg[0:64, :],
                                 start=True, stop=True, tile_position=(0, 0))
                nc.tensor.matmul(ps_w[64:128, 0:64], r_st[64:128, :],
                                 kg[64:128, :], start=True, stop=True,
                                 tile_position=(64, 64))
                wbd = sml.tile([128, 130], f16, tag="wbd", name=f"wbd{p}")
                nc.gpsimd.memset(wbd[0:64, 65:130], 0.0)
                nc.gpsimd.memset(wbd[64:128, 0:65], 0.0)
                nc.gpsimd.memset(wbd[0:64, 64:65], 1.0)
                nc.gpsimd.memset(wbd[64:128, 129:130], 1.0)
                nc.vector.tensor_mul(wbd[0:64, 0:64], ps_w[0:64, 0:64],
                                     rcb[0:64, :].broadcast_to([64, 64]))
                nc.vector.tensor_mul(wbd[64:128, 65:129], ps_w[64:128, 0:64],
                                     rcb[64:128, :].broadcast_to([64, 64]))
                st["wbd"] = wbd

            def x_parts_of(p, st):
                # X phase as a list of small closures so it can thread
                # through the next pair's dense fp16 loop (one psum-group
                # of 2 chunks, or one store DMA, per part).
                a, b = 2 * p, 2 * p + 1
                e1t, wbd = st["e1t"], st["wbd"]
                parts = []

                def mk_group(u, k):
                    def f():
                        if k == 0:
                            st[f"xo{u}"] = med.tile([128, 1024], f32,
                                                    tag="xo",
                                                    name=f"xo{p}_{u}")
                        xo = st[f"xo{u}"]
                        xov = xo.rearrange("p (h bb t d) -> p h bb t d",
                                           h=2, bb=2, t=4)
                        ps_x = psC.tile([128, 512], f32, tag="xinv",
                                        name=f"psx{p}_{u}_{k}")
                        for r in range(2):
                            c = 8 * u + 2 * k + r
                            nc.tensor.matmul(
                                ps_x[:, ds(130 * r, 130)],
                                e1t[:, ds(128 * c, 128)], wbd,
                                start=True, stop=True,
                                skip_group_check=True)
                        psxv = ps_x[:, 0:260].rearrange(
                            "p (r h w) -> p r h w", r=2, h=2)
                        rr = sml.tile([128, 4], f32, tag="rr",
                                      name=f"rr{p}_{u}_{k}")
                        rrv = rr.rearrange("p (r h) -> p r h", r=2)
                        nc.vector.reciprocal(
                            rrv, psxv[:, :, :, 64:65]
                            .rearrange("p r h one -> p r (h one)"))
                        bb, t0 = (2 * k) // 4, (2 * k) % 4
                        nc.vector.tensor_mul(
                            xov[:, :, bb, t0:t0 + 2, :],
                            psxv[:, :, :, 0:64]
                            .rearrange("p r h d -> p h r d"),
                            rrv.rearrange("p r h -> p h r")[:, :, :, None]
                            .broadcast_to([128, 2, 2, 64]))
                    return f

                def mk_store(u, h, sl):
                    def f():
                        xo = st[f"xo{u}"]
                        nc.sync.dma_start(
                            out=xd[sl, ds(1024 * u, 1024), :]
                            .rearrange("(bb p t) d -> p bb (t d)",
                                       bb=2, p=128),
                            in_=xo.rearrange("p (h c) -> p h c", h=2)[:, h]
                            .rearrange("p (bb c) -> p bb c", bb=2))
                    return f

                for u in range(4):
                    for k in range(4):
                        parts.append(mk_group(u, k))
                    parts.append(mk_store(u, 0, a))
                    parts.append(mk_store(u, 1, b))
                return parts

            def emit_x(p, st):
                for f in x_parts_of(p, st):
                    f()

            def emit_e3g_loop(p, st, xparts):
                # dense fp16 stream: E3 groups + E1 groups + (one group
                # late) G matmuls, with the serial NS parts dropped in
                # between so their DVE round trips hide under fp16 MMs.
                st["ps_ga"] = psB.tile([128, 512], f32, tag="gacc",
                                       name=f"psga{p}")
                st["ps_gb"] = psB.tile([128, 512], f32, tag="gacc",
                                       name=f"psgb{p}")
                st["e1t"] = bigT.tile([128, 4096], f16, tag="e1t",
                                      name=f"e1t{p}")
                kts, qts = st["kts"], st["qts"]
                vva, vvb = st["vva"], st["vvb"]
                bdq16, bdk16 = st["bdq16"], st["bdk16"]
                e1t = st["e1t"]

                parts = [lambda: ns_km(p, st)]
                for it in range(6):
                    parts.append(lambda it=it: ns_sq(p, st, it))
                    parts.append(lambda it=it: ns_n3(p, st, it))
                    parts.append(lambda it=it: ns_rn(p, st, it))

                def pop_part():
                    if parts:
                        parts.pop(0)()

                def pop_x():
                    if xparts:
                        xparts.pop(0)()

                def emit_g(g, e3t):
                    gw = 64 if ones_mask else 65
                    for ci in range(4):
                        c = 4 * g + ci
                        first, last = (c == 0), (c == NCHUNK - 1)
                        vcol = 272 * g + 68 * ci  # (bb=g, t=ci) chunk
                        nc.tensor.matmul(
                            st["ps_ga"][0:65, 0:64],
                            vva[:, ds(vcol, 65)],
                            e3t[:, ds(128 * ci, 64)],
                            start=first, stop=last,
                            tile_position=(0, 0), skip_group_check=True)
                        nc.tensor.matmul(
                            st["ps_gb"][0:65, 0:64],
                            vvb[:, ds(vcol, 65)],
                            e3t[:, ds(128 * ci + 64, 64)],
                            start=first, stop=last,
                            tile_position=(0, 0), skip_group_check=True)

                prev_e3t = None
                for g in range(8):
                    ps_e3 = psA.tile([128, 512], f32, tag="bigps",
                                     name=f"pse3{p}_{g}")
                    for ci in range(4):
                        c = 4 * g + ci
                        nc.tensor.matmul(ps_e3[:, ds(128 * ci, 128)],
                                         kts[:, ds(128 * c, 128)], bdq16,
                                         start=True, stop=True,
                                         skip_group_check=True)
                    e3t = med.tile([128, 512], f16, tag="e3t",
                                   name=f"e3t{p}_{g}")
                    nc.scalar.activation(e3t, ps_e3, AF.Exp,
                                         scale=EXP_SCALE_SL)
                    pop_part()
                    pop_x()
                    ps_s1 = psA.tile([128, 512], f32, tag="bigps",
                                     name=f"pss1{p}_{g}")
                    nc.tensor.matmul(ps_s1, bdk16, qts[:, ds(512 * g, 512)],
                                     start=True, stop=True)
                    nc.scalar.activation(e1t[:, ds(512 * g, 512)], ps_s1,
                                         AF.Exp, scale=EXP_SCALE_SL)
                    if prev_e3t is not None:
                        emit_g(g - 1, prev_e3t)
                    prev_e3t = e3t
                    pop_part()
                    pop_x()
                    if g in (3, 5, 7):
                        pop_part()
                    pop_x()
                emit_g(7, prev_e3t)
                while parts:
                    pop_part()
                while xparts:
                    xparts.pop(0)()

            # ---------------- pipelined pair loop -----------------------
            prev = None
            for p in range(npairs):
                st = {"ns_r": None}
                emit_ingest(p, st)
                emit_T(p, st)
                emit_bd(p, st)
                if prev is not None:
                    emit_wchain(prev["p"], prev)
                    emit_kg(prev["p"], prev)
                emit_s2(p, st)
                xparts = []
                if prev is not None:
                    emit_w(prev["p"], prev)
                    xparts = x_parts_of(prev["p"], prev)
                emit_e3g_loop(p, st, xparts)
                st["p"] = p
                prev = st
            emit_wchain(prev["p"], prev)
            emit_kg(prev["p"], prev)
            emit_w(prev["p"], prev)
            emit_x(prev["p"], prev)
    return nc


def _get_program(npairs=NPAIRS, debug=False, ones_mask=True):
    key = (npairs, debug, ones_mask)
    if key not in _PROG_CACHE:
        nc = _build_program(npairs, debug, ones_mask)
        if not nc.is_finalized():
            nc.finalize()  # Bacc defers register allocation until finalize
        _PROG_CACHE[key] = nc
    return _PROG_CACHE[key]


def run(inputs, trace=False, trace_kwargs=None, debug=False):
    from concourse import bass_utils
    Q, K, V, mask = (np.asarray(inputs["Q"], np.float32),
                     np.asarray(inputs["K"], np.float32),
                     np.asarray(inputs["V"], np.float32),
                     np.asarray(inputs["mask"], np.float32))
    ones_mask = bool(mask.min() >= 1.0 and mask.max() <= 1.0)
    rc = np.full((128, 1), 1.0 / _host_global_c(Q, K, mask), np.float32)
    consts16, consts32 = _make_consts()

    if ones_mask:
        Qm, Km, Vm = Q, K, V
    else:
        m = mask[:, None, :, None].astype(np.float32)
        Qm, Km, Vm = Q * m, K * m, V * m

    npair_tot = (B * H) // 2
    # pair-interleaved [48, S, 128]: row s = [T_a[s] | T_b[s]]
    Qp = np.ascontiguousarray(
        Qm.reshape(npair_tot, 2, S, D).transpose(0, 2, 1, 3)
        .reshape(npair_tot, S, 128))
    Kp = np.ascontiguousarray(
        Km.reshape(npair_tot, 2, S, D).transpose(0, 2, 1, 3)
        .reshape(npair_tot, S, 128))
    # V with mask appended as column 64, padded to 68 so each DRAM row
    # is 272B (16B-aligned descriptor starts): [96, S, 68]
    Vx = np.zeros((B * H, S, 68), np.float32)
    Vx[:, :, :64] = Vm.reshape(B * H, S, D)
    Vx[:, :, 64] = np.broadcast_to(mask[:, None, :], (B, H, S)) \
        .reshape(B * H, S)

    nc = _get_program(debug=debug, ones_mask=ones_mask)
    in_maps = []
    for c in range(NCORES):
        in_maps.append({
            "q": Qp[c * NPAIRS:(c + 1) * NPAIRS],
            "k": Kp[c * NPAIRS:(c + 1) * NPAIRS],
            "v": Vx[c * PER_CORE:(c + 1) * PER_CORE],
            "rc": rc,
            "c": consts16,
            "c32": consts32,
        })
    res = bass_utils.run_bass_kernel_spmd(
        nc, in_maps, core_ids=list(range(NCORES)), trace=trace,
        **(trace_kwargs or {}))
    X = np.concatenate([r["x"] for r in res.results], axis=0)
    return X.reshape(B, H, S, D), res


def kernel(**inputs):
    X, _ = run(inputs, trace=False)
    return X


if __name__ == "__main__":
    # quick build check
    prog = _get_program()
    print("built ok")
